# revision 1
# baseline (speedup 1.0000x reference)
"""Causal self-attention (B=2, T=2048, C=1024, H=16, D=64) with RoPE on TRN2.

Sharding: 8 cores = 2 (batch) x 4 (head-groups of 4 heads).
Each core computes qkv projection for its heads, RoPE, causal flash
attention, and a partial o_proj (row-parallel over its heads' dims).
Host gather sums the 4 partial o_proj outputs per batch (the row-parallel
"all-reduce" of the Megatron split) and transposes back to [T, C].

On-chip layout is feature-major (xT = x.T etc.) so every matmul contracts
over the partition dim.  q/k features are de-interleaved (re dims then im
dims per head) by permuting Wqkv columns on the host, which turns RoPE
into 4 block multiplies + 4 block add/subs per 128-row tile.

Attention computes S^T = (K Q^T) tiles directly ([k x q] layout) so the
probabilities come out of exp() already transposed for the P^T @ V
accumulation; softmax denominators are produced by an extra all-ones
column appended to V (row 64 of the PV psum accumulator).  No max
subtraction is needed: logits are O(+-8) here, far below exp overflow.
"""

import sys
import os

sys.path.insert(0, "/opt/trn_rl_repo")

import numpy as np
from contextlib import ExitStack

import concourse.bass as bass
import concourse.bacc as bacc
import concourse.mybir as mybir
import concourse.tile as tile

F32 = mybir.dt.float32
F32R = mybir.dt.float32r

# problem constants (hardcoded per contract)
B, T, C, NH, D = 2, 2048, 1024, 16, 64
HL = 4            # local heads per core
NCORE = 8
CH = 512          # qkv T-chunk width
NCHUNK = T // CH  # 4
QT = 1024         # attention q-tile width
NQT = T // QT     # 2
KB = 128          # attention k-block
SCALE = 1.0 / 8.0  # 1/sqrt(D)

# matmul dtype: float32r streams fp32 at bf16 rate when moving free >= 256
MM_DT = F32R


def _mm(x):
    return x


def _splits(a, b):
    """Split [a, b) at absolute 512 boundaries (psum bank = 512 f32)."""
    out = []
    while a < b:
        nxt = min(b, (a // 512 + 1) * 512)
        out.append((a, nxt))
        a = nxt
    return out


def build_nc():
    nc = bacc.Bacc("TRN2", debug=False, num_devices=NCORE)

    xT_d = nc.dram_tensor("xT", [C, T], F32R, kind="ExternalInput").ap()
    wqk_d = nc.dram_tensor("wqk", [C, 512], F32R, kind="ExternalInput").ap()
    wv_d = nc.dram_tensor("wv", [C, 256], F32R, kind="ExternalInput").ap()
    wo_d = nc.dram_tensor("wo", [256, C], F32R, kind="ExternalInput").ap()
    cosT_d = nc.dram_tensor("cosT", [32, T], F32, kind="ExternalInput").ap()
    sinT_d = nc.dram_tensor("sinT", [32, T], F32, kind="ExternalInput").ap()
    outT_d = nc.dram_tensor("outT", [C, T], F32, kind="ExternalOutput").ap()

    xT_t = xT_d.rearrange("(kt p) t -> kt p t", p=128)    # [8, 128, T]
    wqk_t = wqk_d.rearrange("(kt p) n -> kt p n", p=128)  # [8, 128, 512]
    wv_t = wv_d.rearrange("(kt p) n -> kt p n", p=128)    # [8, 128, 256]
    wo_t = wo_d.rearrange("(kt p) n -> kt p n", p=128)    # [2, 128, C]

    with tile.TileContext(nc) as tc, ExitStack() as ctx:
        const = ctx.enter_context(tc.tile_pool(name="const", bufs=1))
        xcp = ctx.enter_context(tc.tile_pool(name="xcp", bufs=2))
        rtp = ctx.enter_context(tc.tile_pool(name="rtp", bufs=2))
        pp = ctx.enter_context(tc.tile_pool(name="pp", bufs=5))
        nrm = ctx.enter_context(tc.tile_pool(name="nrm", bufs=1))
        psum = ctx.enter_context(tc.tile_pool(name="psum", bufs=2, space="PSUM"))

        # ---- persistent SBUF tensors ----
        # weights in k-block-major single tiles (one big DMA each)
        wqk_all = const.tile([128, 8 * 512], F32R, tag="wqk", name="wqk")
        wqk_v = wqk_all[:].rearrange("p (kt m n) -> p kt m n", m=4, n=128)
        wqk_dv = wqk_d.rearrange("(kt p) (m n) -> p kt m n", p=128, n=128)
        nc.scalar.dma_start(out=wqk_v[:, :, 0], in_=wqk_dv[:, :, 0])
        nc.scalar.dma_start(out=wqk_v[:, :, 1], in_=wqk_dv[:, :, 1])
        wqk_sb = [wqk_all[:, kb * 512:(kb + 1) * 512] for kb in range(8)]

        cc = const.tile([128, T], F32, tag="cc")
        ss = const.tile([128, T], F32, tag="ss")
        nc.scalar.dma_start(out=cc[0:32, :], in_=cosT_d[:])
        nc.scalar.dma_start(out=ss[0:32, :], in_=sinT_d[:])
        nc.scalar.dma_start(out=wqk_v[:, :, 2], in_=wqk_dv[:, :, 2])
        nc.scalar.dma_start(out=wqk_v[:, :, 3], in_=wqk_dv[:, :, 3])
        wv_all = const.tile([128, 8 * 256], F32R, tag="wv", name="wv")
        wv_sb = [wv_all[:, kb * 256:(kb + 1) * 256] for kb in range(8)]

        def load_wv():
            nc.scalar.dma_start(
                out=wv_all[:].rearrange("p (kt n) -> p kt n", n=256),
                in_=wv_d.rearrange("(kt p) n -> p kt n", p=128))
        wo_all = const.tile([128, 2 * C], F32R, tag="wo", name="wo")
        nc.scalar.dma_start(
            out=wo_all[:].rearrange("p (kt n) -> p kt n", n=C),
            in_=wo_d.rearrange("(kt p) n -> p kt n", p=128))
        wo_sb = [wo_all[:, kb * C:(kb + 1) * C] for kb in range(2)]

        # qkT tiles: 0,1 = q (heads 01 / 23), 2,3 = k.  rows per tile:
        # [re_hA(32) im_hA(32) re_hB(32) im_hB(32)] after rope.
        qkT = [const.tile([128, T], F32R, tag=f"qkT{m}", name=f"qkT{m}") for m in range(4)]
        # v tiles, natural layout + ones column per head: [128, 4*65]
        v_sb = [const.tile([128, 4 * 65], F32R, tag=f"v{i}", name=f"v{i}") for i in range(16)]
        ones_f32 = const.tile([128, 1], F32, tag="ones", name="ones")
        nc.gpsimd.memset(ones_f32[:], 1.0)
        for i in range(16):
            ones_ap = v_sb[i][:].rearrange("p (h e) -> p h e", e=65)[:, :, 64]
            nc.vector.tensor_copy(ones_ap, ones_f32[:, 0:1].to_broadcast((128, 4)))
        # y^T tiles: [128, T] x2 (4 heads x 64 dims)
        yT = [const.tile([128, T], F32R, tag=f"yT{kb}", name=f"yT{kb}") for kb in range(2)]

        chunk_xc = {}

        def qkv_chunk(n):
            qkv_chunk_qk(n)
            qkv_chunk_v(n)

        def qkv_chunk_qk(n):
            t0 = n * CH
            xc_all = xcp.tile([128, 8 * CH], F32R, tag="xc", name="xc")
            xc_view = xc_all[:].rearrange("p (kt t) -> p kt t", t=CH)
            xd_view = xT_d[:, t0:t0 + CH].rearrange("(kt p) t -> p kt t", p=128)
            nc.sync.dma_start(out=xc_view[:, 0:4], in_=xd_view[:, 0:4])
            nc.sync.dma_start(out=xc_view[:, 4:8], in_=xd_view[:, 4:8])
            xc = [xc_all[:, kb * CH:(kb + 1) * CH] for kb in range(8)]
            chunk_xc[n] = xc
            # q/k m-tiles: m0 = re dims of all 4 q heads, m1 = im dims,
            # m2/m3 same for k.  rope = 6 full-width DVE ops per q/k,
            # then 16 small copies relayout to head-contiguous qkT.
            mul = mybir.AluOpType.mult
            sub = mybir.AluOpType.subtract
            add = mybir.AluOpType.add
            rep = nc.gpsimd
            for r in range(1, 4):
                rep.tensor_copy(
                    cc[32 * r:32 * r + 32, t0:t0 + CH], cc[0:32, t0:t0 + CH])
                rep.tensor_copy(
                    ss[32 * r:32 * r + 32, t0:t0 + CH], ss[0:32, t0:t0 + CH])
            for g in range(2):  # 0 = q, 1 = k
                pre = psum.tile([128, CH], F32, tag="qk", name="psre")
                pim = psum.tile([128, CH], F32, tag="qk", name="psim")
                for ps, m in ((pre, 2 * g), (pim, 2 * g + 1)):
                    for kb in range(8):
                        nc.tensor.matmul(
                            ps[:, 0:CH],
                            lhsT=_mm(wqk_sb[kb][:, m * 128:(m + 1) * 128]),
                            rhs=_mm(xc[kb]),
                            start=(kb == 0),
                            stop=(kb == 7),
                        )
                ccn = cc[:, t0:t0 + CH]
                ssn = ss[:, t0:t0 + CH]
                t1 = rtp.tile([128, CH], F32, tag="t1")
                t2 = rtp.tile([128, CH], F32, tag="t2")
                t3 = rtp.tile([128, CH], F32, tag="t3")
                t4 = rtp.tile([128, CH], F32, tag="t4")
                nc.vector.tensor_tensor(t1[:], pre[:, 0:CH], ccn, mul)
                nc.vector.tensor_tensor(t2[:], pim[:, 0:CH], ssn, mul)
                nc.vector.tensor_tensor(t3[:], pre[:, 0:CH], ssn, mul)
                nc.vector.tensor_tensor(t4[:], pim[:, 0:CH], ccn, mul)
                rall, iall = t1, t3
                nc.vector.tensor_tensor(rall[:], t1[:], t2[:], sub)
                nc.vector.tensor_tensor(iall[:], t3[:], t4[:], add)
                # relayout: head h -> qkT[2*g + h//2] rows 64*(h%2)+[re|im]
                for h in range(4):
                    o = qkT[2 * g + h // 2]
                    r0 = 64 * (h % 2)
                    if h == 2:
                        nc.scalar.copy(
                            o[r0:r0 + 32, t0:t0 + CH], rall[32 * h:32 * h + 32, :])
                        nc.scalar.copy(
                            o[r0 + 32:r0 + 64, t0:t0 + CH], iall[32 * h:32 * h + 32, :])
                        continue
                    eng = nc.vector if h == 0 else nc.gpsimd
                    eng.tensor_copy(
                        o[r0:r0 + 32, t0:t0 + CH], rall[32 * h:32 * h + 32, :])
                    eng.tensor_copy(
                        o[r0 + 32:r0 + 64, t0:t0 + CH], iall[32 * h:32 * h + 32, :])


        def qkv_chunk_v(n):
            t0 = n * CH
            xc = chunk_xc[n]
            # v: natural layout [T-part, d]
            for tb in range(4):
                psv = psum.tile([128, CH], F32, tag="qk", name="psv")
                for kb in range(8):
                    nc.tensor.matmul(
                        psv[:, 0:256],
                        lhsT=_mm(xc[kb][:, tb * 128:(tb + 1) * 128]),
                        rhs=_mm(wv_sb[kb]),
                        start=(kb == 0),
                        stop=(kb == 7),
                    )
                vt = v_sb[4 * n + tb]
                dst = vt[:].rearrange("p (h e) -> p h e", e=65)[:, :, 0:64]
                src = psv[:, 0:256].rearrange("p (h d) -> p h d", d=64)
                nc.scalar.copy(dst, src)

        def attention(h, qt):
            """One head, one q-tile of width QT."""
            q0 = qt * QT
            qtile = qkT[h // 2]
            ktile = qkT[2 + h // 2]
            r0 = 64 * (h % 2)
            psy = psum.tile([65, QT], F32, tag="y", name="psy", bufs=1)
            nkb = 8 * qt + 8
            for kb in range(nkb):
                diag = kb >= 8 * qt
                off = 128 * (kb - 8 * qt) if diag else 0
                pst = psum.tile([128, QT], F32, tag="st", name="pst")
                for (a, b) in _splits(off, QT):
                    nc.tensor.matmul(
                        pst[:, a:b],
                        lhsT=_mm(ktile[r0:r0 + 64, kb * 128:(kb + 1) * 128]),
                        rhs=_mm(qtile[r0:r0 + 64, q0 + a:q0 + b]),
                        start=True,
                        stop=True,
                    )
                P = pp.tile([128, QT], F32R, tag="P")
                nc.scalar.activation(
                    P[:, off:QT], pst[:, off:QT],
                    mybir.ActivationFunctionType.Exp, scale=SCALE)
                if diag:
                    # zero strictly-upper triangle of the leading 128 cols
                    nc.gpsimd.affine_select(
                        out=P[:, off:off + 128],
                        in_=P[:, off:off + 128],
                        compare_op=mybir.AluOpType.is_ge,
                        fill=0.0,
                        base=0,
                        pattern=[[1, 128]],
                        channel_multiplier=-1,
                    )
                for (a, b) in _splits(off, QT):
                    # last writer of psum bank r is diag j = 4r+3
                    j_stop = 4 * (a // 512) + 3
                    nc.tensor.matmul(
                        psy[:, a:b],
                        lhsT=_mm(v_sb[kb][:, h * 65:h * 65 + 65]),
                        rhs=_mm(P[:, a:b]),
                        start=(kb == 0),
                        stop=(diag and (kb - 8 * qt) == j_stop),
                    )
            ybuf = nrm.tile([64, QT], F32, tag="ybuf", bufs=2)
            lrow = nrm.tile([1, QT], F32, tag="lrow", bufs=1)
            rl = nrm.tile([1, QT], F32, tag="rl", bufs=1)
            rlb = nrm.tile([64, QT], F32, tag="rlb", bufs=2)
            # bounce psy to SBUF (frees the psum slot early).  The custom-DVE
            # reciprocal misreads inputs at non-zero partition base on HW, so
            # the denominator row must land in a partition-0 tile.
            nc.vector.tensor_copy(ybuf[:], psy[0:64, :])
            nc.vector.tensor_copy(lrow[:], psy[64:65, :])
            nc.vector.reciprocal_approx_fast(rl[:], lrow[:])
            nc.gpsimd.partition_broadcast(rlb[:], rl[:])
            nc.vector.tensor_tensor(
                yT[h // 2][r0:r0 + 64, q0:q0 + QT],
                ybuf[:], rlb[:], mybir.AluOpType.mult)

        def o_proj(ntp):
            for mo in range(8):
                    ps = psum.tile([128, QT], F32, tag="st")
                    for half in range(2):
                        nt = ntp * 2 + half
                        for kb in range(2):
                            nc.tensor.matmul(
                                ps[:, half * 512:(half + 1) * 512],
                                lhsT=_mm(wo_sb[kb][:, mo * 128:(mo + 1) * 128]),
                                rhs=_mm(yT[kb][:, nt * 512:(nt + 1) * 512]),
                                start=(kb == 0),
                                stop=(kb == 1),
                            )
                    ob = pp.tile([128, QT], F32, tag="P", name="ob")
                    # tail pass: ACT's serial finish costs ~2us; keep the
                    # last copies on DVE which is idle by then
                    on_act = (mo % 2 == 1) and not (ntp == 1 and mo >= 4)
                    if on_act:
                        nc.scalar.copy(ob[:], ps[:])
                    else:
                        nc.vector.tensor_copy(ob[:], ps[:])
                    ring = nc.scalar if (ntp == 1 and mo % 2 == 1) else nc.sync
                    ring.dma_start(
                        out=outT_d[mo * 128:(mo + 1) * 128,
                                   ntp * QT:(ntp + 1) * QT],
                        in_=ob[:])

        load_wv()
        qkv_chunk(0)
        qkv_chunk(1)
        for h in range(HL):
            attention(h, 0)
        qkv_chunk(2)
        qkv_chunk(3)
        o_proj(0)
        for h in range(HL):
            attention(h, 1)
        o_proj(1)

    nc.compile()
    return nc


def shard_inputs(x, freqs_cos, freqs_sin, Wqkv, Wo):
    """Build the 8 per-core input maps (host-side sharding)."""
    x = np.asarray(x, dtype=np.float32)
    Wqkv = np.asarray(Wqkv, dtype=np.float32)
    Wo = np.asarray(Wo, dtype=np.float32)
    cosT = np.ascontiguousarray(np.asarray(freqs_cos, dtype=np.float32).T)
    sinT = np.ascontiguousarray(np.asarray(freqs_sin, dtype=np.float32).T)
    xTs = [np.ascontiguousarray(x[b].T) for b in range(B)]

    in_maps = []
    for c in range(NCORE):
        b, hg = c // 4, c % 4
        re = [np.arange(g * 64, g * 64 + 64, 2)
              for g in range(4 * hg, 4 * hg + 4)]
        im = [np.arange(g * 64 + 1, g * 64 + 64, 2)
              for g in range(4 * hg, 4 * hg + 4)]
        qcols = np.concatenate(re + im)
        kcols = C + qcols
        wqk = np.ascontiguousarray(Wqkv[:, np.concatenate([qcols, kcols])])
        wv = np.ascontiguousarray(Wqkv[:, 2 * C + hg * 256: 2 * C + hg * 256 + 256])
        wo = np.ascontiguousarray(Wo[hg * 256: hg * 256 + 256, :])
        in_maps.append({
            "xT": xTs[b], "wqk": wqk, "wv": wv, "wo": wo,
            "cosT": cosT, "sinT": sinT,
        })
    return in_maps


_NC_CACHE = None


def _get_nc():
    global _NC_CACHE
    if _NC_CACHE is None:
        _NC_CACHE = build_nc()
    return _NC_CACHE


def run(inputs, trace=False):
    from concourse.bass_utils import run_bass_kernel_spmd

    nc = _get_nc()
    in_maps = shard_inputs(**inputs)
    res = run_bass_kernel_spmd(nc, in_maps, list(range(NCORE)), trace=trace)
    out = np.empty((B, T, C), dtype=np.float32)
    for b in range(B):
        acc = res.results[4 * b]["outT"].astype(np.float32)
        for c in range(4 * b + 1, 4 * b + 4):
            acc = acc + res.results[c]["outT"]
        out[b] = acc.T
    return out, res


def kernel(**inputs):
    out, _ = run(inputs)
    return out



# revision 25
# speedup vs baseline: 1.2244x; 1.2244x over previous
"""Causal self-attention (B=2, T=2048, C=1024, H=16, D=64) with RoPE on TRN2.

Sharding: 8 cores = 2 (batch) x 4 (head-groups of 4 heads).
Each core: qkv projection for its heads (fp16), RoPE, causal attention
(fp16 matmuls, fp32 psum), partial o_proj (row-parallel).  Host sums the
4 partial outputs per batch.

Layout is feature-major (xT etc.) so matmuls contract over partitions.
q/k features are de-interleaved on the host (re dims then im dims per
head) so RoPE is 4 full-width mults + sub/add per 128-row tile, then 8
small fp16 copies (4x DVE mode) relayout to head-contiguous qkT.

Attention computes S^T = (K Q^T) per 128-row k-block so probabilities
exit exp() already transposed for P^T @ V.  V tiles carry 64 all-ones
columns: the PV matmul then yields psum rows 0-63 = y, rows 64-127 = the
softmax denominator replicated 64x (zero extra PE cycles since matmul
cost is free-dim only) -- normalization is one DVE reciprocal + one DVE
multiply, no partition broadcast.  No max subtraction: logits are O(+-8)
and exp fits fp16 range.

q-tiles are staged 512/512/1024 so exp (ACT) starts as soon as chunk 0's
qkv is done; qkv chunks 2/3 and o_proj interleave into the attention
stream to keep PE busy while ACT chews exps.
"""

import sys
import os

sys.path.insert(0, "/opt/trn_rl_repo")

import numpy as np
from contextlib import ExitStack

import concourse.bass as bass
import concourse.bacc as bacc
import concourse.mybir as mybir
import concourse.tile as tile

F32 = mybir.dt.float32
F16 = mybir.dt.float16

# problem constants (hardcoded per contract)
B, T, C, NH, D = 2, 2048, 1024, 16, 64
HL = 4            # local heads per core
NCORE = 8
CH = 512          # qkv T-chunk width
NCHUNK = T // CH  # 4
SCALE = 1.0 / 8.0  # 1/sqrt(D)
NKB = T // 128    # 16 k-blocks


def _splits(a, b):
    """Split [a, b) at 512 boundaries (psum bank = 512 f32)."""
    out = []
    while a < b:
        nxt = min(b, (a // 512 + 1) * 512)
        out.append((a, nxt))
        a = nxt
    return out


def build_nc():
    nc = bacc.Bacc("TRN2", debug=False, num_devices=NCORE)

    xT_d = nc.dram_tensor("xT", [C, T], F16, kind="ExternalInput").ap()
    wqk_d = nc.dram_tensor("wqk", [C, 512], F16, kind="ExternalInput").ap()
    wv_d = nc.dram_tensor("wv", [C, 256], F16, kind="ExternalInput").ap()
    wo_d = nc.dram_tensor("wo", [256, C], F16, kind="ExternalInput").ap()
    ccT_d = nc.dram_tensor("ccT", [128, T], F16, kind="ExternalInput").ap()
    ssT_d = nc.dram_tensor("ssT", [128, T], F16, kind="ExternalInput").ap()
    outT_d = nc.dram_tensor("outT", [C, T], F16, kind="ExternalOutput").ap()

    with tile.TileContext(nc) as tc, ExitStack() as ctx:
        const = ctx.enter_context(tc.tile_pool(name="const", bufs=1))
        xcp = ctx.enter_context(tc.tile_pool(name="xcp", bufs=2))
        rtp = ctx.enter_context(tc.tile_pool(name="rtp", bufs=2))
        pp = ctx.enter_context(tc.tile_pool(name="pp", bufs=4))
        nrm = ctx.enter_context(tc.tile_pool(name="nrm", bufs=2))
        obp = ctx.enter_context(tc.tile_pool(name="obp", bufs=3))
        psum = ctx.enter_context(tc.tile_pool(name="psum", bufs=2, space="PSUM"))

        # ---- persistent SBUF tensors ----
        # wqk in two DMAs: q columns (m0/m1) land first so the first real
        # matmuls can start while the k half is still in flight.
        wqk_all = const.tile([128, 8 * 512], F16, tag="wqk", name="wqk")
        wqk_v = wqk_all[:].rearrange("p (kt n) -> p kt n", n=512)
        wqk_dv = wqk_d.rearrange("(kt p) n -> p kt n", p=128)
        nc.scalar.dma_start(out=wqk_v[:, :, 0:256], in_=wqk_dv[:, :, 0:256])
        nc.scalar.dma_start(out=wqk_v[:, :, 256:512], in_=wqk_dv[:, :, 256:512])
        wqk_sb = [wqk_all[:, kb * 512:(kb + 1) * 512] for kb in range(8)]

        # remaining input DMAs spread across issue queues so nothing
        # serializes behind the wqk/x loads
        cc = const.tile([128, T], F16, tag="cc")
        ss = const.tile([128, T], F16, tag="ss")
        nc.scalar.dma_start(out=cc[:], in_=ccT_d[:])
        nc.scalar.dma_start(out=ss[:], in_=ssT_d[:])

        wv_all = const.tile([128, 8 * 256], F16, tag="wv", name="wv")
        nc.scalar.dma_start(
            out=wv_all[:].rearrange("p (kt n) -> p kt n", n=256),
            in_=wv_d.rearrange("(kt p) n -> p kt n", p=128))
        wv_sb = [wv_all[:, kb * 256:(kb + 1) * 256] for kb in range(8)]

        # PE warm-up: dependency-free matmuls that cover the input-DMA wait
        # and carry the tensor engine through its p-state ramp before the
        # first real matmul issues.
        warm = const.tile([128, 512], F16, tag="warm")
        nc.gpsimd.memset(warm[:], 0.0)
        pw = psum.tile([128, 512], F32, tag="qk", name="pw")
        for _ in range(20):
            nc.tensor.matmul(
                pw[:, 0:512], lhsT=warm[:, 0:128], rhs=warm[:],
                start=True, stop=True)

        wo_all = const.tile([128, 2 * C], F16, tag="wo", name="wo")
        nc.scalar.dma_start(
            out=wo_all[:].rearrange("p (kt n) -> p kt n", n=C),
            in_=wo_d.rearrange("(kt p) n -> p kt n", p=128))
        wo_sb = [wo_all[:, kb * C:(kb + 1) * C] for kb in range(2)]

        # qkT tiles: 0,1 = q (heads 01 / 23), 2,3 = k.  rows per tile:
        # [re_hA(32) im_hA(32) re_hB(32) im_hB(32)] after rope.
        qkT = [const.tile([128, T], F16, tag=f"qkT{m}", name=f"qkT{m}")
               for m in range(4)]
        # v: [128 kpos, 4 heads x 16 blocks x 128] fp16; cols 0-63 of each
        # block = v dims, cols 64-127 = ones (denominator rows of PV psum)
        vT = const.tile([128, HL * NKB * 128], F16, tag="vT", name="vT")
        vT_v = vT[:].rearrange("p (h b c) -> p h b c", h=HL, b=NKB)
        nc.gpsimd.memset(vT_v[:, :, :, 64:128], 1.0)
        # y^T tiles: [128, T] x2 (4 heads x 64 dims)
        yT = [const.tile([128, T], F16, tag=f"yT{kb}", name=f"yT{kb}")
              for kb in range(2)]

        chunk_xc = {}

        def x_load(n):
            t0 = n * CH
            xc_all = xcp.tile([128, 8 * CH], F16, tag="xc", name="xc")
            xc_view = xc_all[:].rearrange("p (kt t) -> p kt t", t=CH)
            xd_view = xT_d[:, t0:t0 + CH].rearrange("(kt p) t -> p kt t", p=128)
            nc.sync.dma_start(out=xc_view[:, 0:4], in_=xd_view[:, 0:4])
            nc.sync.dma_start(out=xc_view[:, 4:8], in_=xd_view[:, 4:8])
            chunk_xc[n] = [xc_all[:, kb * CH:(kb + 1) * CH] for kb in range(8)]

        def qk_g(n, g):
            """q (g=0) or k (g=1) projection + rope for chunk n."""
            t0 = n * CH
            xc = chunk_xc[n]
            mul = mybir.AluOpType.mult
            sub = mybir.AluOpType.subtract
            add = mybir.AluOpType.add
            pre = psum.tile([128, CH], F32, tag="qk", name="psre")
            pim = psum.tile([128, CH], F32, tag="qk", name="psim")
            for ps, m in ((pre, 2 * g), (pim, 2 * g + 1)):
                for kb in range(8):
                    nc.tensor.matmul(
                        ps[:, 0:CH],
                        lhsT=wqk_sb[kb][:, m * 128:(m + 1) * 128],
                        rhs=xc[kb],
                        start=(kb == 0),
                        stop=(kb == 7),
                    )
            # psum -> fp16 bounce (keeps ACT free for exp -- ACT paces the
            # attention phases); the rope arithmetic then runs in DVE 2x
            # fp16 mode which more than pays for the extra copy.
            preb = rtp.tile([128, CH], F16, tag="preb")
            pimb = rtp.tile([128, CH], F16, tag="pimb")
            nc.scalar.copy(preb[:], pre[:, 0:CH])
            nc.scalar.copy(pimb[:], pim[:, 0:CH])
            ccn = cc[:, t0:t0 + CH]
            ssn = ss[:, t0:t0 + CH]
            t1 = rtp.tile([128, CH], F16, tag="t1")
            t2 = rtp.tile([128, CH], F16, tag="t2")
            t3 = rtp.tile([128, CH], F16, tag="t3")
            t4 = rtp.tile([128, CH], F16, tag="t4")
            nc.vector.tensor_tensor(t1[:], preb[:], ccn, mul)
            nc.vector.tensor_tensor(t2[:], pimb[:], ssn, mul)
            nc.vector.tensor_tensor(t3[:], preb[:], ssn, mul)
            nc.vector.tensor_tensor(t4[:], pimb[:], ccn, mul)
            rall, iall = t1, t3
            nc.vector.tensor_tensor(rall[:], t1[:], t2[:], sub)
            nc.vector.tensor_tensor(iall[:], t3[:], t4[:], add)
            # relayout: head h -> qkT[2*g + h//2] rows 64*(h%2)+[re|im]
            # fp16 sbuf-to-sbuf copies run in 4x DVE mode.
            for h in range(4):
                o = qkT[2 * g + h // 2]
                r0 = 64 * (h % 2)
                nc.vector.tensor_copy(
                    o[r0:r0 + 32, t0:t0 + CH], rall[32 * h:32 * h + 32, :])
                nc.vector.tensor_copy(
                    o[r0 + 32:r0 + 64, t0:t0 + CH],
                    iall[32 * h:32 * h + 32, :])

        def v_tb(n, tb):
            xc = chunk_xc[n]
            psv = psum.tile([128, CH], F32, tag="qk", name="psv")
            for kb in range(8):
                nc.tensor.matmul(
                    psv[:, 0:256],
                    lhsT=xc[kb][:, tb * 128:(tb + 1) * 128],
                    rhs=wv_sb[kb],
                    start=(kb == 0),
                    stop=(kb == 7),
                )
            blk = 4 * n + tb
            dst = vT_v[:, :, blk, 0:64]
            src = psv[:, 0:256].rearrange("p (h d) -> p h d", d=64)
            # gpsimd cannot read PSUM; DVE does the psum->fp16 bounce
            nc.vector.tensor_copy(dst, src)

        filler_q = []

        def pump():
            if filler_q:
                filler_q.pop(0)()

        def attention(h, q0, qn, pump_every=0):
            """One head, q-cols [q0, q0+qn).  k-blocks 0..(q0+qn)/128."""
            qtile = qkT[h // 2]
            ktile = qkT[2 + h // 2]
            r0 = 64 * (h % 2)
            nkb = (q0 + qn) // 128
            fd = q0 // 128  # first diagonal block
            psy = psum.tile([128, qn], F32, tag="y", name="psy", bufs=1)
            Ps = {}

            def block_off(kb):
                return 128 * (kb - fd) if kb >= fd else 0

            def scores_block(kb):
                off = block_off(kb)
                pst = psum.tile([128, qn], F32, tag="st", name="pst")
                for (a, b) in _splits(off, qn):
                    nc.tensor.matmul(
                        pst[:, a:b],
                        lhsT=ktile[r0:r0 + 64, kb * 128:(kb + 1) * 128],
                        rhs=qtile[r0:r0 + 64, q0 + a:q0 + b],
                        start=True,
                        stop=True,
                    )
                P = pp.tile([128, 1024], F16, tag="P")
                Ps[kb] = P
                nc.scalar.activation(
                    P[:, off:qn], pst[:, off:qn],
                    mybir.ActivationFunctionType.Exp, scale=SCALE)
                if kb >= fd:
                    # zero strictly-upper triangle of the leading 128 cols
                    nc.gpsimd.affine_select(
                        out=P[:, off:off + 128],
                        in_=P[:, off:off + 128],
                        compare_op=mybir.AluOpType.is_ge,
                        fill=0.0,
                        base=0,
                        pattern=[[1, 128]],
                        channel_multiplier=-1,
                    )

            def pv_block(kb):
                off = block_off(kb)
                P = Ps.pop(kb)
                for (a, b) in _splits(off, qn):
                    # last writer of the psum bank holding col a is diag
                    # block fd + 4*(a//512) + 3
                    kb_stop = min(fd + 4 * (a // 512) + 3, nkb - 1)
                    nc.tensor.matmul(
                        psy[:, a:b],
                        lhsT=vT_v[:, h, kb, :],
                        rhs=P[:, a:b],
                        start=(kb == 0),
                        stop=(kb == kb_stop),
                    )

            def normalize(a, b):
                # psum rows 64-127 all hold the denominator row l (ones
                # cols of vT): reciprocal + one multiply per psum bank,
                # emitted as soon as that bank's accumulation closes.
                rlb = nrm.tile([64, 512], F32, tag="rlb")
                nc.vector.reciprocal(rlb[:, 0:b - a], psy[64:128, a:b])
                nc.vector.tensor_tensor(
                    yT[h // 2][r0:r0 + 64, q0 + a:q0 + b],
                    psy[0:64, a:b], rlb[:, 0:b - a], mybir.AluOpType.mult)

            # scores run one block ahead of PV so PE is not stalled on exp
            for kb in range(nkb):
                scores_block(kb)
                if pump_every and kb % pump_every == 0:
                    pump()
                if kb > 0:
                    pv_block(kb - 1)
                    if kb - 1 == min(fd + 3, nkb - 1) and qn > 512:
                        normalize(0, 512)  # bank 0 closed early
            pv_block(nkb - 1)
            normalize(512 if qn > 512 else 0, qn)

        def o_proj(nt, mo, tail=False):
            """Output block: feat rows [128*mo ..+128), q [512*nt ..+512)."""
            ob = obp.tile([128, 512], F16, tag="ob", name="ob")
            ps = psum.tile([128, CH], F32, tag="qk", name="psob")
            for kb in range(2):
                nc.tensor.matmul(
                    ps[:, 0:512],
                    lhsT=wo_sb[kb][:, mo * 128:(mo + 1) * 128],
                    rhs=yT[kb][:, nt * 512:(nt + 1) * 512],
                    start=(kb == 0),
                    stop=(kb == 1),
                )
            # in the tail ACT is idle once the exps are done: it takes half
            # the psum bounces there
            if tail and mo % 2 == 1:
                nc.scalar.copy(ob[:], ps[:, 0:512])
            else:
                nc.vector.tensor_copy(ob[:], ps[:, 0:512])
            # keep DMA issue off the ACT queue while exps run; in the tail
            # ACT is free and a second queue doubles drain bandwidth
            ring = nc.scalar if (tail and mo % 2 == 1) else nc.sync
            ring.dma_start(
                out=outT_d[mo * 128:(mo + 1) * 128, nt * 512:(nt + 1) * 512],
                in_=ob[:])

        # ---- schedule ----
        # Emission order == per-engine queue order.  Attention is the only
        # ACT (exp) consumer and PE outpaces ACT ~2:1 there, so qkv/o_proj
        # work is pumped into the attention stream as PE filler at k-block
        # granularity.  Dependency safety comes from emission order: a
        # filler is always emitted before the instruction that needs it.
        x_load(0)
        x_load(1)
        qk_g(0, 0)
        qk_g(0, 1)
        for tb in range(4):
            v_tb(0, tb)
        # tile A (q 0-512): needs only chunk 0.  fillers: chunk 1 qk + v.
        filler_q.extend([
            lambda: qk_g(1, 0),
            lambda: qk_g(1, 1),
            lambda: v_tb(1, 0),
            lambda: v_tb(1, 1),
            lambda: v_tb(1, 2),
            lambda: v_tb(1, 3),
        ])
        attention(0, 0, 512, pump_every=2)
        attention(1, 0, 512, pump_every=2)
        attention(2, 0, 512, pump_every=2)
        attention(3, 0, 512, pump_every=2)
        while filler_q:
            pump()
        # tile B (q 512-1024): fillers: chunks 2,3 qk.
        x_load(2)
        filler_q.extend([
            lambda: qk_g(2, 0),
            lambda: qk_g(2, 1),
            lambda: x_load(3),
            lambda: qk_g(3, 0),
            lambda: qk_g(3, 1),
        ])
        attention(0, 512, 512, pump_every=3)
        attention(1, 512, 512, pump_every=3)
        attention(2, 512, 512, pump_every=3)
        attention(3, 512, 512, pump_every=3)
        while filler_q:
            pump()
        # tile C (q 1024-2048): fillers: chunk 2/3 v (needed from block 8 /
        # 12 of the first head) and o_proj of q 0-1024.
        filler_q.extend([lambda tb=tb: v_tb(2, tb) for tb in range(4)])
        filler_q.extend([lambda tb=tb: v_tb(3, tb) for tb in range(4)])
        filler_q.extend(
            (lambda nt=nt, mo=mo: o_proj(nt, mo))
            for nt in range(2) for mo in range(8))
        attention(0, 1024, 1024, pump_every=2)
        attention(1, 1024, 1024, pump_every=2)
        attention(2, 1024, 1024, pump_every=2)
        attention(3, 1024, 1024, pump_every=2)
        while filler_q:
            pump()
        for nt in (2, 3):
            for mo in range(8):
                o_proj(nt, mo, tail=True)

    nc.compile()
    return nc


def shard_inputs(x, freqs_cos, freqs_sin, Wqkv, Wo):
    """Build the 8 per-core input maps (host-side sharding)."""
    x = np.asarray(x, dtype=np.float32)
    Wqkv = np.asarray(Wqkv, dtype=np.float32)
    Wo = np.asarray(Wo, dtype=np.float32)
    # cos/sin tables transposed and replicated x4 (one copy per local head)
    ccT = np.tile(np.asarray(freqs_cos, dtype=np.float32).T, (4, 1))
    ssT = np.tile(np.asarray(freqs_sin, dtype=np.float32).T, (4, 1))
    ccT = np.ascontiguousarray(ccT).astype(np.float16)
    ssT = np.ascontiguousarray(ssT).astype(np.float16)
    xTs = [np.ascontiguousarray(x[b].T).astype(np.float16) for b in range(B)]

    in_maps = []
    for c in range(NCORE):
        b, hg = c // 4, c % 4
        re = [np.arange(g * 64, g * 64 + 64, 2)
              for g in range(4 * hg, 4 * hg + 4)]
        im = [np.arange(g * 64 + 1, g * 64 + 64, 2)
              for g in range(4 * hg, 4 * hg + 4)]
        qcols = np.concatenate(re + im)
        kcols = C + qcols
        wqk = np.ascontiguousarray(
            Wqkv[:, np.concatenate([qcols, kcols])]).astype(np.float16)
        wv = np.ascontiguousarray(
            Wqkv[:, 2 * C + hg * 256: 2 * C + hg * 256 + 256]).astype(np.float16)
        wo = np.ascontiguousarray(
            Wo[hg * 256: hg * 256 + 256, :]).astype(np.float16)
        in_maps.append({
            "xT": xTs[b], "wqk": wqk, "wv": wv, "wo": wo,
            "ccT": ccT, "ssT": ssT,
        })
    return in_maps


_NC_CACHE = None


def _get_nc():
    global _NC_CACHE
    if _NC_CACHE is None:
        _NC_CACHE = build_nc()
    return _NC_CACHE


def run(inputs, trace=False):
    from concourse.bass_utils import run_bass_kernel_spmd

    nc = _get_nc()
    in_maps = shard_inputs(**inputs)
    res = run_bass_kernel_spmd(nc, in_maps, list(range(NCORE)), trace=trace)
    out = np.empty((B, T, C), dtype=np.float32)
    for b in range(B):
        acc = res.results[4 * b]["outT"].astype(np.float32)
        for c in range(4 * b + 1, 4 * b + 4):
            acc = acc + res.results[c]["outT"].astype(np.float32)
        out[b] = acc.T
    return out, res


def kernel(**inputs):
    out, _ = run(inputs)
    return out


# revision 38
# speedup vs baseline: 1.2873x; 1.0514x over previous
"""Causal self-attention (B=2, T=2048, C=1024, H=16, D=64) with RoPE on TRN2.

Sharding: 8 cores = 2 (batch) x 4 (head-groups of 4 heads).
Each core: qkv projection for its heads (fp16), RoPE, causal attention
(fp16 matmuls, fp32 psum), partial o_proj (row-parallel).  Host sums the
4 partial outputs per batch.

Layout is feature-major (xT etc.) so matmuls contract over partitions.
q/k features are de-interleaved on the host (re dims then im dims per
head) so RoPE is 4 full-width mults + sub/add per 128-row tile, then 8
small fp16 copies (4x DVE mode) relayout to head-contiguous qkT.

Attention computes S^T = (K Q^T) per 128-row k-block so probabilities
exit exp() already transposed for P^T @ V.  V tiles carry 64 all-ones
columns: the PV matmul then yields psum rows 0-63 = y, rows 64-127 = the
softmax denominator replicated 64x (zero extra PE cycles since matmul
cost is free-dim only) -- normalization is one DVE reciprocal + one DVE
multiply, no partition broadcast.  No max subtraction: logits are O(+-8)
and exp fits fp16 range.

q-tiles are staged 512/512/1024 so exp (ACT) starts as soon as chunk 0's
qkv is done; qkv chunks 2/3 and o_proj interleave into the attention
stream to keep PE busy while ACT chews exps.
"""

import sys
import os

sys.path.insert(0, "/opt/trn_rl_repo")

import numpy as np
from contextlib import ExitStack

import concourse.bass as bass
import concourse.bacc as bacc
import concourse.mybir as mybir
import concourse.tile as tile

F32 = mybir.dt.float32
F16 = mybir.dt.float16

# problem constants (hardcoded per contract)
B, T, C, NH, D = 2, 2048, 1024, 16, 64
HL = 4            # local heads per core
NCORE = 8
CH = 512          # qkv T-chunk width
NCHUNK = T // CH  # 4
SCALE = 1.0 / 8.0  # 1/sqrt(D)
NKB = T // 128    # 16 k-blocks


def _splits(a, b):
    """Split [a, b) at 512 boundaries (psum bank = 512 f32)."""
    out = []
    while a < b:
        nxt = min(b, (a // 512 + 1) * 512)
        out.append((a, nxt))
        a = nxt
    return out


def build_nc():
    nc = bacc.Bacc("TRN2", debug=False, num_devices=NCORE)

    xT_d = nc.dram_tensor("xT", [C, T], F16, kind="ExternalInput").ap()
    wqk_d = nc.dram_tensor("wqk", [C, 512], F16, kind="ExternalInput").ap()
    wv_d = nc.dram_tensor("wv", [C, 256], F16, kind="ExternalInput").ap()
    wo_d = nc.dram_tensor("wo", [256, C], F16, kind="ExternalInput").ap()
    ccT_d = nc.dram_tensor("ccT", [128, T], F16, kind="ExternalInput").ap()
    ssT_d = nc.dram_tensor("ssT", [128, T], F16, kind="ExternalInput").ap()
    outT_d = nc.dram_tensor("outT", [C, T], F16, kind="ExternalOutput").ap()

    with tile.TileContext(nc) as tc, ExitStack() as ctx:
        const = ctx.enter_context(tc.tile_pool(name="const", bufs=1))
        xcp = ctx.enter_context(tc.tile_pool(name="xcp", bufs=2))
        rtp = ctx.enter_context(tc.tile_pool(name="rtp", bufs=2))
        pp = ctx.enter_context(tc.tile_pool(name="pp", bufs=4))
        nrm = ctx.enter_context(tc.tile_pool(name="nrm", bufs=2))
        obp = ctx.enter_context(tc.tile_pool(name="obp", bufs=3))
        psum = ctx.enter_context(tc.tile_pool(name="psum", bufs=2, space="PSUM"))

        # ---- persistent SBUF tensors ----
        # wqk in two DMAs: q columns (m0/m1) land first so the first real
        # matmuls can start while the k half is still in flight.
        wqk_all = const.tile([128, 8 * 512], F16, tag="wqk", name="wqk")
        wqk_v = wqk_all[:].rearrange("p (kt n) -> p kt n", n=512)
        wqk_dv = wqk_d.rearrange("(kt p) n -> p kt n", p=128)
        nc.scalar.dma_start(out=wqk_v[:, :, 0:256], in_=wqk_dv[:, :, 0:256])
        nc.scalar.dma_start(out=wqk_v[:, :, 256:512], in_=wqk_dv[:, :, 256:512])
        wqk_sb = [wqk_all[:, kb * 512:(kb + 1) * 512] for kb in range(8)]

        # remaining input DMAs spread across issue queues so nothing
        # serializes behind the wqk/x loads
        cc = const.tile([128, T], F16, tag="cc")
        ss = const.tile([128, T], F16, tag="ss")
        nc.scalar.dma_start(out=cc[:], in_=ccT_d[:])
        nc.scalar.dma_start(out=ss[:], in_=ssT_d[:])

        wv_all = const.tile([128, 8 * 256], F16, tag="wv", name="wv")
        nc.scalar.dma_start(
            out=wv_all[:].rearrange("p (kt n) -> p kt n", n=256),
            in_=wv_d.rearrange("(kt p) n -> p kt n", p=128))
        wv_sb = [wv_all[:, kb * 256:(kb + 1) * 256] for kb in range(8)]

        # PE warm-up: dependency-free matmuls that cover the input-DMA wait
        # and carry the tensor engine through its p-state ramp before the
        # first real matmul issues.
        warm = const.tile([128, 512], F16, tag="warm")
        nc.gpsimd.memset(warm[:], 0.0)
        pw = psum.tile([128, 512], F32, tag="qk", name="pw")
        for _ in range(14):
            nc.tensor.matmul(
                pw[:, 0:512], lhsT=warm[:, 0:128], rhs=warm[:],
                start=True, stop=True)

        wo_all = const.tile([128, 2 * C], F16, tag="wo", name="wo")
        nc.sync.dma_start(
            out=wo_all[:].rearrange("p (kt n) -> p kt n", n=C),
            in_=wo_d.rearrange("(kt p) n -> p kt n", p=128))
        wo_sb = [wo_all[:, kb * C:(kb + 1) * C] for kb in range(2)]

        # qkT tiles: 0,1 = q (heads 01 / 23), 2,3 = k.  rows per tile:
        # [re_hA(32) im_hA(32) re_hB(32) im_hB(32)] after rope.
        qkT = [const.tile([128, T], F16, tag=f"qkT{m}", name=f"qkT{m}")
               for m in range(4)]
        # v: [128 kpos, 4 heads x 16 blocks x 128] fp16; cols 0-63 of each
        # block = v dims, cols 64-127 = ones (denominator rows of PV psum)
        vT = const.tile([128, HL * NKB * 128], F16, tag="vT", name="vT")
        vT_v = vT[:].rearrange("p (h b c) -> p h b c", h=HL, b=NKB)
        nc.gpsimd.memset(vT_v[:, :, :, 64:128], 1.0)
        # y^T tiles: [128, T] x2 (4 heads x 64 dims)
        yT = [const.tile([128, T], F16, tag=f"yT{kb}", name=f"yT{kb}")
              for kb in range(2)]

        chunk_xc = {}

        def x_load(n):
            t0 = n * CH
            xc_all = xcp.tile([128, 8 * CH], F16, tag="xc", name="xc")
            xc_view = xc_all[:].rearrange("p (kt t) -> p kt t", t=CH)
            xd_view = xT_d[:, t0:t0 + CH].rearrange("(kt p) t -> p kt t", p=128)
            nc.sync.dma_start(out=xc_view[:, 0:4], in_=xd_view[:, 0:4])
            nc.sync.dma_start(out=xc_view[:, 4:8], in_=xd_view[:, 4:8])
            chunk_xc[n] = [xc_all[:, kb * CH:(kb + 1) * CH] for kb in range(8)]

        def qk_g(n, g):
            """q (g=0) or k (g=1) projection + rope for chunk n."""
            t0 = n * CH
            xc = chunk_xc[n]
            mul = mybir.AluOpType.mult
            sub = mybir.AluOpType.subtract
            add = mybir.AluOpType.add
            pre = psum.tile([128, CH], F32, tag="qk", name="psre")
            pim = psum.tile([128, CH], F32, tag="qk", name="psim")
            for ps, m in ((pre, 2 * g), (pim, 2 * g + 1)):
                for kb in range(8):
                    nc.tensor.matmul(
                        ps[:, 0:CH],
                        lhsT=wqk_sb[kb][:, m * 128:(m + 1) * 128],
                        rhs=xc[kb],
                        start=(kb == 0),
                        stop=(kb == 7),
                    )
            ccn = cc[:, t0:t0 + CH]
            ssn = ss[:, t0:t0 + CH]
            t1 = rtp.tile([128, CH], F16, tag="t1")
            t2 = rtp.tile([128, CH], F16, tag="t2")
            t3 = rtp.tile([128, CH], F16, tag="t3")
            t4 = rtp.tile([128, CH], F16, tag="t4")
            # psum -> fp16 bounce on ACT (slack in the A/B phases; also
            # frees the psum slots quickly for the next projection); the
            # rope arithmetic then runs in DVE 2x fp16 mode.
            preb = rtp.tile([128, CH], F16, tag="preb")
            pimb = rtp.tile([128, CH], F16, tag="pimb")
            nc.scalar.copy(preb[:], pre[:, 0:CH])
            nc.scalar.copy(pimb[:], pim[:, 0:CH])
            preb, pimb = preb[:], pimb[:]
            nc.vector.tensor_tensor(t1[:], preb, ccn, mul)
            nc.vector.tensor_tensor(t2[:], pimb, ssn, mul)
            nc.vector.tensor_tensor(t3[:], preb, ssn, mul)
            nc.vector.tensor_tensor(t4[:], pimb, ccn, mul)
            rall, iall = t1, t3
            nc.vector.tensor_tensor(rall[:], t1[:], t2[:], sub)
            nc.vector.tensor_tensor(iall[:], t3[:], t4[:], add)
            # relayout: head h -> qkT[2*g + h//2] rows 64*(h%2)+[re|im]
            # fp16 sbuf-to-sbuf copies run in 4x DVE mode; Pool (idle in
            # these phases) takes a share to flatten the DVE bursts.
            for h in range(4):
                o = qkT[2 * g + h // 2]
                r0 = 64 * (h % 2)
                eng = nc.gpsimd if h == 3 else nc.vector
                eng.tensor_copy(
                    o[r0:r0 + 32, t0:t0 + CH], rall[32 * h:32 * h + 32, :])
                eng.tensor_copy(
                    o[r0 + 32:r0 + 64, t0:t0 + CH],
                    iall[32 * h:32 * h + 32, :])

        def v_tb(n, tb):
            xc = chunk_xc[n]
            psv = psum.tile([128, CH], F32, tag="qk", name="psv")
            for kb in range(8):
                nc.tensor.matmul(
                    psv[:, 0:256],
                    lhsT=xc[kb][:, tb * 128:(tb + 1) * 128],
                    rhs=wv_sb[kb],
                    start=(kb == 0),
                    stop=(kb == 7),
                )
            blk = 4 * n + tb
            dst = vT_v[:, :, blk, 0:64]
            src = psv[:, 0:256].rearrange("p (h d) -> p h d", d=64)
            # gpsimd cannot read PSUM; DVE does the psum->fp16 bounce
            nc.vector.tensor_copy(dst, src)

        filler_q = []

        def pump():
            if filler_q:
                filler_q.pop(0)()

        def attention(h, q0, qn, pump_every=0, at_blocks=None):
            """One head, q-cols [q0, q0+qn).  k-blocks 0..(q0+qn)/128.

            at_blocks: {kb: [unit, ...]} -- mandatory work units emitted
            just before scores_block(kb); used for dependencies of later
            pv_blocks (e.g. v tiles), unlike best-effort pump fillers.
            """
            qtile = qkT[h // 2]
            ktile = qkT[2 + h // 2]
            r0 = 64 * (h % 2)
            nkb = (q0 + qn) // 128
            fd = q0 // 128  # first diagonal block
            psy = psum.tile([128, qn], F32, tag="y", name="psy", bufs=1)
            Ps = {}

            def block_off(kb):
                return 128 * (kb - fd) if kb >= fd else 0

            def scores_block(kb):
                off = block_off(kb)
                pst = psum.tile([128, qn], F32, tag="st", name="pst")
                for (a, b) in _splits(off, qn):
                    nc.tensor.matmul(
                        pst[:, a:b],
                        lhsT=ktile[r0:r0 + 64, kb * 128:(kb + 1) * 128],
                        rhs=qtile[r0:r0 + 64, q0 + a:q0 + b],
                        start=True,
                        stop=True,
                    )
                P = pp.tile([128, 1024], F16, tag="P")
                Ps[kb] = P
                nc.scalar.activation(
                    P[:, off:qn], pst[:, off:qn],
                    mybir.ActivationFunctionType.Exp, scale=SCALE)
                if kb >= fd:
                    # zero strictly-upper triangle of the leading 128 cols
                    nc.gpsimd.affine_select(
                        out=P[:, off:off + 128],
                        in_=P[:, off:off + 128],
                        compare_op=mybir.AluOpType.is_ge,
                        fill=0.0,
                        base=0,
                        pattern=[[1, 128]],
                        channel_multiplier=-1,
                    )

            def pv_block(kb):
                off = block_off(kb)
                P = Ps.pop(kb)
                for (a, b) in _splits(off, qn):
                    # last writer of the psum bank holding col a is diag
                    # block fd + 4*(a//512) + 3
                    kb_stop = min(fd + 4 * (a // 512) + 3, nkb - 1)
                    nc.tensor.matmul(
                        psy[:, a:b],
                        lhsT=vT_v[:, h, kb, :],
                        rhs=P[:, a:b],
                        start=(kb == 0),
                        stop=(kb == kb_stop),
                    )

            def normalize(a, b):
                # psum rows 64-127 all hold the denominator row l (ones
                # cols of vT): reciprocal + one multiply per psum bank,
                # emitted as soon as that bank's accumulation closes.
                rlb = nrm.tile([64, 512], F32, tag="rlb")
                nc.vector.reciprocal(rlb[:, 0:b - a], psy[64:128, a:b])
                nc.vector.tensor_tensor(
                    yT[h // 2][r0:r0 + 64, q0 + a:q0 + b],
                    psy[0:64, a:b], rlb[:, 0:b - a], mybir.AluOpType.mult)

            # scores run one block ahead of PV so PE is not stalled on exp
            for kb in range(nkb):
                if at_blocks and kb in at_blocks:
                    for u in at_blocks[kb]:
                        u()
                scores_block(kb)
                if pump_every and kb % pump_every == 0:
                    pump()
                if kb > 0:
                    pv_block(kb - 1)
                    if kb - 1 == min(fd + 3, nkb - 1) and qn > 512:
                        normalize(0, 512)  # bank 0 closed early
            pv_block(nkb - 1)
            normalize(512 if qn > 512 else 0, qn)

        def o_proj(nt, mo, tail=False):
            """Output block: feat rows [128*mo ..+128), q [512*nt ..+512)."""
            ob = obp.tile([128, 512], F16, tag="ob", name="ob")
            ps = psum.tile([128, CH], F32, tag="qk", name="psob")
            for kb in range(2):
                nc.tensor.matmul(
                    ps[:, 0:512],
                    lhsT=wo_sb[kb][:, mo * 128:(mo + 1) * 128],
                    rhs=yT[kb][:, nt * 512:(nt + 1) * 512],
                    start=(kb == 0),
                    stop=(kb == 1),
                )
            # in the tail ACT is idle once the exps are done: it takes half
            # the psum bounces there
            if tail and mo % 2 == 1:
                nc.scalar.copy(ob[:], ps[:, 0:512])
            else:
                nc.vector.tensor_copy(ob[:], ps[:, 0:512])
            # keep DMA issue off the ACT queue while exps run; in the tail
            # ACT is free and a second queue doubles drain bandwidth
            ring = nc.scalar if (tail and mo % 2 == 1) else nc.sync
            ring.dma_start(
                out=outT_d[mo * 128:(mo + 1) * 128, nt * 512:(nt + 1) * 512],
                in_=ob[:])

        # ---- schedule ----
        # Emission order == per-engine queue order.  Attention is the only
        # ACT (exp) consumer and PE outpaces ACT ~2:1 there, so qkv/o_proj
        # work is pumped into the attention stream as PE filler at k-block
        # granularity.  Dependency safety comes from emission order: a
        # filler is always emitted before the instruction that needs it.
        x_load(0)
        x_load(1)
        qk_g(0, 0)
        qk_g(0, 1)
        for tb in range(4):
            v_tb(0, tb)
        v_tb(1, 0)
        v_tb(1, 1)
        qk_g(1, 0)
        # tile A (q 0-512): needs only chunk 0.  fillers: rest of chunk 1.
        filler_q.extend([
            lambda: qk_g(1, 1),
            lambda: v_tb(1, 2),
            lambda: v_tb(1, 3),
        ])
        attention(0, 0, 512, pump_every=2)
        attention(1, 0, 512, pump_every=2)
        attention(2, 0, 512, pump_every=2)
        attention(3, 0, 512, pump_every=2)
        while filler_q:
            pump()
        # tile B (q 512-1024): fillers: chunks 2,3 qk.
        x_load(2)
        qk_g(2, 0)
        filler_q.extend([
            lambda: qk_g(2, 1),
            lambda: x_load(3),
            lambda: qk_g(3, 0),
            lambda: qk_g(3, 1),
        ])
        attention(0, 512, 512, pump_every=3)
        attention(1, 512, 512, pump_every=3)
        attention(2, 512, 512, pump_every=3)
        attention(3, 512, 512, pump_every=3)
        while filler_q:
            pump()
        # tile C (q 1024-2048): v chunks 2/3 are emitted at fixed blocks of
        # the first head (hard deps of pv blocks 8-15); o_proj of q 0-1024
        # is order-free filler spread across all four heads.
        filler_q.extend(
            (lambda nt=nt, mo=mo: o_proj(nt, mo))
            for nt in range(2) for mo in range(8))
        attention(0, 1024, 1024, at_blocks={
            5: [lambda: v_tb(2, 0), lambda: v_tb(2, 1)],
            7: [lambda: v_tb(2, 2), lambda: v_tb(2, 3)],
            9: [lambda: v_tb(3, 0), lambda: v_tb(3, 1)],
            11: [lambda: v_tb(3, 2), lambda: v_tb(3, 3)],
        })
        attention(1, 1024, 1024, pump_every=3)
        attention(2, 1024, 1024, pump_every=3)
        attention(3, 1024, 1024, pump_every=3)
        while filler_q:
            pump()
        # tail: q 1024-2048.  per mo one [128, 1024] ob filled in two
        # halves (nt=2 as soon as the early-closed psum bank is
        # normalized, nt=3 after the last), one merged DMA per mo.
        tail_obs = {}

        def tail_half(nt, mo):
            if mo not in tail_obs:
                tail_obs[mo] = obp.tile([128, 1024], F16, tag="obt",
                                        name="obt", bufs=8)
            ob = tail_obs[mo]
            ps = psum.tile([128, CH], F32, tag="qk", name="psob")
            for kb in range(2):
                nc.tensor.matmul(
                    ps[:, 0:512],
                    lhsT=wo_sb[kb][:, mo * 128:(mo + 1) * 128],
                    rhs=yT[kb][:, nt * 512:(nt + 1) * 512],
                    start=(kb == 0),
                    stop=(kb == 1),
                )
            half = nt - 2
            if mo % 2 == 1:
                nc.scalar.copy(ob[:, half * 512:(half + 1) * 512],
                               ps[:, 0:512])
            else:
                nc.vector.tensor_copy(
                    ob[:, half * 512:(half + 1) * 512], ps[:, 0:512])
            if nt == 3:
                ring = nc.scalar if mo % 2 == 1 else nc.sync
                ring.dma_start(
                    out=outT_d[mo * 128:(mo + 1) * 128, 1024:2048],
                    in_=ob[:])

        for mo in range(8):
            tail_half(2, mo)
        for mo in range(8):
            tail_half(3, mo)

    nc.compile()
    return nc


def shard_inputs(x, freqs_cos, freqs_sin, Wqkv, Wo):
    """Build the 8 per-core input maps (host-side sharding)."""
    x = np.asarray(x, dtype=np.float32)
    Wqkv = np.asarray(Wqkv, dtype=np.float32)
    Wo = np.asarray(Wo, dtype=np.float32)
    # cos/sin tables transposed and replicated x4 (one copy per local head)
    ccT = np.tile(np.asarray(freqs_cos, dtype=np.float32).T, (4, 1))
    ssT = np.tile(np.asarray(freqs_sin, dtype=np.float32).T, (4, 1))
    ccT = np.ascontiguousarray(ccT).astype(np.float16)
    ssT = np.ascontiguousarray(ssT).astype(np.float16)
    xTs = [np.ascontiguousarray(x[b].T).astype(np.float16) for b in range(B)]

    in_maps = []
    for c in range(NCORE):
        b, hg = c // 4, c % 4
        re = [np.arange(g * 64, g * 64 + 64, 2)
              for g in range(4 * hg, 4 * hg + 4)]
        im = [np.arange(g * 64 + 1, g * 64 + 64, 2)
              for g in range(4 * hg, 4 * hg + 4)]
        qcols = np.concatenate(re + im)
        kcols = C + qcols
        wqk = np.ascontiguousarray(
            Wqkv[:, np.concatenate([qcols, kcols])]).astype(np.float16)
        wv = np.ascontiguousarray(
            Wqkv[:, 2 * C + hg * 256: 2 * C + hg * 256 + 256]).astype(np.float16)
        wo = np.ascontiguousarray(
            Wo[hg * 256: hg * 256 + 256, :]).astype(np.float16)
        in_maps.append({
            "xT": xTs[b], "wqk": wqk, "wv": wv, "wo": wo,
            "ccT": ccT, "ssT": ssT,
        })
    return in_maps


_NC_CACHE = None


def _get_nc():
    global _NC_CACHE
    if _NC_CACHE is None:
        _NC_CACHE = build_nc()
    return _NC_CACHE


def run(inputs, trace=False):
    from concourse.bass_utils import run_bass_kernel_spmd

    nc = _get_nc()
    in_maps = shard_inputs(**inputs)
    res = run_bass_kernel_spmd(nc, in_maps, list(range(NCORE)), trace=trace)
    out = np.empty((B, T, C), dtype=np.float32)
    for b in range(B):
        acc = res.results[4 * b]["outT"].astype(np.float32)
        for c in range(4 * b + 1, 4 * b + 4):
            acc = acc + res.results[c]["outT"].astype(np.float32)
        out[b] = acc.T
    return out, res


def kernel(**inputs):
    out, _ = run(inputs)
    return out


# revision 49
# speedup vs baseline: 1.2897x; 1.0018x over previous
"""Causal self-attention (B=2, T=2048, C=1024, H=16, D=64) with RoPE on TRN2.

Sharding: 8 cores = 2 (batch) x 4 (head-groups of 4 heads).
Each core: qkv projection for its heads (fp16), RoPE, causal attention
(fp16 matmuls, fp32 psum), partial o_proj (row-parallel).  Host sums the
4 partial outputs per batch.

Layout is feature-major (xT etc.) so matmuls contract over partitions.
q/k features are de-interleaved on the host (re dims then im dims per
head) so RoPE is 4 full-width mults + sub/add per 128-row tile, then 8
small fp16 copies (4x DVE mode) relayout to head-contiguous qkT.

Attention computes S^T = (K Q^T) per 128-row k-block so probabilities
exit exp() already transposed for P^T @ V.  V tiles carry 64 all-ones
columns: the PV matmul then yields psum rows 0-63 = y, rows 64-127 = the
softmax denominator replicated 64x (zero extra PE cycles since matmul
cost is free-dim only) -- normalization is one DVE reciprocal + one DVE
multiply, no partition broadcast.  No max subtraction: logits are O(+-8)
and exp fits fp16 range.

q-tiles are staged 512/512/1024 so exp (ACT) starts as soon as chunk 0's
qkv is done; qkv chunks 2/3 and o_proj interleave into the attention
stream to keep PE busy while ACT chews exps.
"""

import sys
import os

sys.path.insert(0, "/opt/trn_rl_repo")

import numpy as np
from contextlib import ExitStack

import concourse.bass as bass
import concourse.bacc as bacc
import concourse.mybir as mybir
import concourse.tile as tile

F32 = mybir.dt.float32
F16 = mybir.dt.float16

# problem constants (hardcoded per contract)
B, T, C, NH, D = 2, 2048, 1024, 16, 64
HL = 4            # local heads per core
NCORE = 8
CH = 512          # qkv T-chunk width
NCHUNK = T // CH  # 4
SCALE = 1.0 / 8.0  # 1/sqrt(D)
NKB = T // 128    # 16 k-blocks


def _splits(a, b):
    """Split [a, b) at 512 boundaries (psum bank = 512 f32)."""
    out = []
    while a < b:
        nxt = min(b, (a // 512 + 1) * 512)
        out.append((a, nxt))
        a = nxt
    return out


def build_nc():
    nc = bacc.Bacc("TRN2", debug=False, num_devices=NCORE)

    xT_d = nc.dram_tensor("xT", [C, T], F16, kind="ExternalInput").ap()
    wqk_d = nc.dram_tensor("wqk", [C, 512], F16, kind="ExternalInput").ap()
    wv_d = nc.dram_tensor("wv", [C, 256], F16, kind="ExternalInput").ap()
    wo_d = nc.dram_tensor("wo", [256, C], F16, kind="ExternalInput").ap()
    ccT_d = nc.dram_tensor("ccT", [128, T], F16, kind="ExternalInput").ap()
    ssT_d = nc.dram_tensor("ssT", [128, T], F16, kind="ExternalInput").ap()
    outT_d = nc.dram_tensor("outT", [C, T], F16, kind="ExternalOutput").ap()

    with tile.TileContext(nc) as tc, ExitStack() as ctx:
        const = ctx.enter_context(tc.tile_pool(name="const", bufs=1))
        xcp = ctx.enter_context(tc.tile_pool(name="xcp", bufs=2))
        rtp = ctx.enter_context(tc.tile_pool(name="rtp", bufs=2))
        pp = ctx.enter_context(tc.tile_pool(name="pp", bufs=4))
        nrm = ctx.enter_context(tc.tile_pool(name="nrm", bufs=2))
        obp = ctx.enter_context(tc.tile_pool(name="obp", bufs=3))
        psum = ctx.enter_context(tc.tile_pool(name="psum", bufs=2, space="PSUM"))

        # ---- persistent SBUF tensors ----
        # wqk in two DMAs: q columns (m0/m1) land first so the first real
        # matmuls can start while the k half is still in flight.
        wqk_all = const.tile([128, 8 * 512], F16, tag="wqk", name="wqk")
        wqk_v = wqk_all[:].rearrange("p (kt n) -> p kt n", n=512)
        wqk_dv = wqk_d.rearrange("(kt p) n -> p kt n", p=128)
        nc.scalar.dma_start(out=wqk_v[:, :, 0:256], in_=wqk_dv[:, :, 0:256])
        nc.scalar.dma_start(out=wqk_v[:, :, 256:512], in_=wqk_dv[:, :, 256:512])
        wqk_sb = [wqk_all[:, kb * 512:(kb + 1) * 512] for kb in range(8)]

        # remaining input DMAs spread across issue queues so nothing
        # serializes behind the wqk/x loads
        cc = const.tile([128, T], F16, tag="cc")
        ss = const.tile([128, T], F16, tag="ss")
        nc.scalar.dma_start(out=cc[:], in_=ccT_d[:])
        nc.scalar.dma_start(out=ss[:], in_=ssT_d[:])

        wv_all = const.tile([128, 8 * 256], F16, tag="wv", name="wv")
        nc.scalar.dma_start(
            out=wv_all[:].rearrange("p (kt n) -> p kt n", n=256),
            in_=wv_d.rearrange("(kt p) n -> p kt n", p=128))
        wv_sb = [wv_all[:, kb * 256:(kb + 1) * 256] for kb in range(8)]

        # PE warm-up: dependency-free matmuls that cover the input-DMA wait
        # and carry the tensor engine through its p-state ramp before the
        # first real matmul issues.
        warm = const.tile([128, 512], F16, tag="warm")
        nc.gpsimd.memset(warm[:], 0.0)
        pw = psum.tile([128, 512], F32, tag="qk", name="pw")
        for _ in range(14):
            nc.tensor.matmul(
                pw[:, 0:512], lhsT=warm[:, 0:128], rhs=warm[:],
                start=True, stop=True)

        wo_all = const.tile([128, 2 * C], F16, tag="wo", name="wo")
        nc.sync.dma_start(
            out=wo_all[:].rearrange("p (kt n) -> p kt n", n=C),
            in_=wo_d.rearrange("(kt p) n -> p kt n", p=128))
        wo_sb = [wo_all[:, kb * C:(kb + 1) * C] for kb in range(2)]

        # qkT tiles: 0,1 = q (heads 01 / 23), 2,3 = k.  rows per tile:
        # [re_hA(32) im_hA(32) re_hB(32) im_hB(32)] after rope.
        qkT = [const.tile([128, T], F16, tag=f"qkT{m}", name=f"qkT{m}")
               for m in range(4)]
        # v: [128 kpos, 4 heads x 16 blocks x 128] fp16; cols 0-63 of each
        # block = v dims, cols 64-127 = ones (denominator rows of PV psum)
        vT = const.tile([128, HL * NKB * 128], F16, tag="vT", name="vT")
        vT_v = vT[:].rearrange("p (h b c) -> p h b c", h=HL, b=NKB)
        nc.gpsimd.memset(vT_v[:, :, :, 64:128], 1.0)
        # y^T tiles: [128, T] x2 (4 heads x 64 dims)
        yT = [const.tile([128, T], F16, tag=f"yT{kb}", name=f"yT{kb}")
              for kb in range(2)]

        chunk_xc = {}

        def x_load(n):
            t0 = n * CH
            xc_all = xcp.tile([128, 8 * CH], F16, tag="xc", name="xc")
            xc_view = xc_all[:].rearrange("p (kt t) -> p kt t", t=CH)
            xd_view = xT_d[:, t0:t0 + CH].rearrange("(kt p) t -> p kt t", p=128)
            nc.sync.dma_start(out=xc_view[:, 0:4], in_=xd_view[:, 0:4])
            nc.sync.dma_start(out=xc_view[:, 4:8], in_=xd_view[:, 4:8])
            chunk_xc[n] = [xc_all[:, kb * CH:(kb + 1) * CH] for kb in range(8)]

        def qk_g(n, g):
            """q (g=0) or k (g=1) projection + rope for chunk n."""
            t0 = n * CH
            xc = chunk_xc[n]
            mul = mybir.AluOpType.mult
            sub = mybir.AluOpType.subtract
            add = mybir.AluOpType.add
            pre = psum.tile([128, CH], F32, tag="qk", name="psre")
            pim = psum.tile([128, CH], F32, tag="qk", name="psim")
            for ps, m in ((pre, 2 * g), (pim, 2 * g + 1)):
                for kb in range(8):
                    nc.tensor.matmul(
                        ps[:, 0:CH],
                        lhsT=wqk_sb[kb][:, m * 128:(m + 1) * 128],
                        rhs=xc[kb],
                        start=(kb == 0),
                        stop=(kb == 7),
                    )
            ccn = cc[:, t0:t0 + CH]
            ssn = ss[:, t0:t0 + CH]
            t1 = rtp.tile([128, CH], F16, tag="t1")
            t2 = rtp.tile([128, CH], F16, tag="t2")
            t3 = rtp.tile([128, CH], F16, tag="t3")
            t4 = rtp.tile([128, CH], F16, tag="t4")
            # psum -> fp16 bounce on ACT (slack in the A/B phases; also
            # frees the psum slots quickly for the next projection); the
            # rope arithmetic then runs in DVE 2x fp16 mode.
            preb = rtp.tile([128, CH], F16, tag="preb")
            pimb = rtp.tile([128, CH], F16, tag="pimb")
            nc.scalar.copy(preb[:], pre[:, 0:CH])
            nc.scalar.copy(pimb[:], pim[:, 0:CH])
            preb, pimb = preb[:], pimb[:]
            nc.vector.tensor_tensor(t1[:], preb, ccn, mul)
            nc.vector.tensor_tensor(t2[:], pimb, ssn, mul)
            nc.vector.tensor_tensor(t3[:], preb, ssn, mul)
            nc.vector.tensor_tensor(t4[:], pimb, ccn, mul)
            rall, iall = t1, t3
            nc.vector.tensor_tensor(rall[:], t1[:], t2[:], sub)
            nc.vector.tensor_tensor(iall[:], t3[:], t4[:], add)
            # relayout: head h -> qkT[2*g + h//2] rows 64*(h%2)+[re|im]
            # fp16 sbuf-to-sbuf copies run in 4x DVE mode; Pool (idle in
            # these phases) takes a share to flatten the DVE bursts.
            for h in range(4):
                o = qkT[2 * g + h // 2]
                r0 = 64 * (h % 2)
                eng = nc.gpsimd if h == 3 else nc.vector
                eng.tensor_copy(
                    o[r0:r0 + 32, t0:t0 + CH], rall[32 * h:32 * h + 32, :])
                eng.tensor_copy(
                    o[r0 + 32:r0 + 64, t0:t0 + CH],
                    iall[32 * h:32 * h + 32, :])

        def v_tb(n, tb):
            xc = chunk_xc[n]
            psv = psum.tile([128, CH], F32, tag="qk", name="psv")
            for kb in range(8):
                nc.tensor.matmul(
                    psv[:, 0:256],
                    lhsT=xc[kb][:, tb * 128:(tb + 1) * 128],
                    rhs=wv_sb[kb],
                    start=(kb == 0),
                    stop=(kb == 7),
                )
            blk = 4 * n + tb
            dst = vT_v[:, :, blk, 0:64]
            src = psv[:, 0:256].rearrange("p (h d) -> p h d", d=64)
            # gpsimd cannot read PSUM; DVE does the psum->fp16 bounce
            nc.vector.tensor_copy(dst, src)

        filler_q = []

        def dummy(n=2):
            # keep-warm matmuls: PE p-state drops 2x after an idle gap and
            # needs 3us of continuous execution to recover; padding known
            # exp-bound stretches keeps the real matmuls at full clock.
            pd = psum.tile([128, 512], F32, tag="qk", name="pd")
            for _ in range(n):
                nc.tensor.matmul(
                    pd[:, 0:512], lhsT=warm[:, 0:128], rhs=warm[:],
                    start=True, stop=True)

        def pump(keep_warm=0):
            if filler_q:
                filler_q.pop(0)()
            elif keep_warm:
                dummy(keep_warm)

        def attention(h, q0, qn, pump_every=0, at_blocks=None,
                      keep_warm=0):
            """One head, q-cols [q0, q0+qn).  k-blocks 0..(q0+qn)/128.

            at_blocks: {kb: [unit, ...]} -- mandatory work units emitted
            just before scores_block(kb); used for dependencies of later
            pv_blocks (e.g. v tiles), unlike best-effort pump fillers.
            """
            qtile = qkT[h // 2]
            ktile = qkT[2 + h // 2]
            r0 = 64 * (h % 2)
            nkb = (q0 + qn) // 128
            fd = q0 // 128  # first diagonal block
            psy = psum.tile([128, qn], F32, tag="y", name="psy", bufs=1)
            Ps = {}

            def block_off(kb):
                return 128 * (kb - fd) if kb >= fd else 0

            def scores_block(kb):
                off = block_off(kb)
                pst = psum.tile([128, qn], F32, tag="st", name="pst")
                for (a, b) in _splits(off, qn):
                    nc.tensor.matmul(
                        pst[:, a:b],
                        lhsT=ktile[r0:r0 + 64, kb * 128:(kb + 1) * 128],
                        rhs=qtile[r0:r0 + 64, q0 + a:q0 + b],
                        start=True,
                        stop=True,
                    )
                P = pp.tile([128, 1024], F16, tag="P")
                Ps[kb] = P
                nc.scalar.activation(
                    P[:, off:qn], pst[:, off:qn],
                    mybir.ActivationFunctionType.Exp, scale=SCALE)
                if kb >= fd:
                    # zero strictly-upper triangle of the leading 128 cols
                    nc.gpsimd.affine_select(
                        out=P[:, off:off + 128],
                        in_=P[:, off:off + 128],
                        compare_op=mybir.AluOpType.is_ge,
                        fill=0.0,
                        base=0,
                        pattern=[[1, 128]],
                        channel_multiplier=-1,
                    )

            def pv_block(kb):
                off = block_off(kb)
                P = Ps.pop(kb)
                for (a, b) in _splits(off, qn):
                    # last writer of the psum bank holding col a is diag
                    # block fd + 4*(a//512) + 3
                    kb_stop = min(fd + 4 * (a // 512) + 3, nkb - 1)
                    nc.tensor.matmul(
                        psy[:, a:b],
                        lhsT=vT_v[:, h, kb, :],
                        rhs=P[:, a:b],
                        start=(kb == 0),
                        stop=(kb == kb_stop),
                    )

            def normalize(a, b):
                # psum rows 64-127 all hold the denominator row l (ones
                # cols of vT): reciprocal + one multiply per psum bank,
                # emitted as soon as that bank's accumulation closes.
                rlb = nrm.tile([64, 512], F32, tag="rlb")
                nc.vector.reciprocal(rlb[:, 0:b - a], psy[64:128, a:b])
                nc.vector.tensor_tensor(
                    yT[h // 2][r0:r0 + 64, q0 + a:q0 + b],
                    psy[0:64, a:b], rlb[:, 0:b - a], mybir.AluOpType.mult)

            # scores run one block ahead of PV so PE is not stalled on exp
            for kb in range(nkb):
                if at_blocks and kb in at_blocks:
                    for u in at_blocks[kb]:
                        u()
                if pump_every and kb % pump_every == 0:
                    pump(keep_warm)
                scores_block(kb)
                if kb > 0:
                    pv_block(kb - 1)
                    if kb - 1 == min(fd + 3, nkb - 1) and qn > 512:
                        normalize(0, 512)  # bank 0 closed early
            pv_block(nkb - 1)
            normalize(512 if qn > 512 else 0, qn)

        def o_proj(nt, mo, tail=False):
            """Output block: feat rows [128*mo ..+128), q [512*nt ..+512)."""
            ob = obp.tile([128, 512], F16, tag="ob", name="ob")
            ps = psum.tile([128, CH], F32, tag="qk", name="psob")
            for kb in range(2):
                nc.tensor.matmul(
                    ps[:, 0:512],
                    lhsT=wo_sb[kb][:, mo * 128:(mo + 1) * 128],
                    rhs=yT[kb][:, nt * 512:(nt + 1) * 512],
                    start=(kb == 0),
                    stop=(kb == 1),
                )
            # in the tail ACT is idle once the exps are done: it takes half
            # the psum bounces there
            if tail and mo % 2 == 1:
                nc.scalar.copy(ob[:], ps[:, 0:512])
            else:
                nc.vector.tensor_copy(ob[:], ps[:, 0:512])
            # keep DMA issue off the ACT queue while exps run; in the tail
            # ACT is free and a second queue doubles drain bandwidth
            ring = nc.scalar if (tail and mo % 2 == 1) else nc.sync
            ring.dma_start(
                out=outT_d[mo * 128:(mo + 1) * 128, nt * 512:(nt + 1) * 512],
                in_=ob[:])

        # ---- schedule ----
        # Emission order == per-engine queue order.  Attention is the only
        # ACT (exp) consumer and PE outpaces ACT ~2:1 there, so qkv/o_proj
        # work is pumped into the attention stream as PE filler at k-block
        # granularity.  Dependency safety comes from emission order: a
        # filler is always emitted before the instruction that needs it.
        x_load(0)
        x_load(1)
        qk_g(0, 0)
        qk_g(0, 1)
        for tb in range(4):
            v_tb(0, tb)
        v_tb(1, 0)
        v_tb(1, 1)
        qk_g(1, 0)
        # tile A (q 0-512): needs only chunk 0.  fillers: rest of chunk 1.
        filler_q.extend([
            lambda: qk_g(1, 1),
            lambda: v_tb(1, 2),
            lambda: v_tb(1, 3),
        ])
        attention(0, 0, 512, pump_every=2)
        attention(1, 0, 512, pump_every=2)
        attention(2, 0, 512, pump_every=2)
        attention(3, 0, 512, pump_every=2)
        while filler_q:
            pump()
        # tile B (q 512-1024): fillers: chunks 2,3 qk.
        x_load(2)
        qk_g(2, 0)
        filler_q.extend([
            lambda: qk_g(2, 1),
            lambda: x_load(3),
            lambda: qk_g(3, 0),
            lambda: qk_g(3, 1),
        ])
        attention(0, 512, 512, pump_every=3)
        attention(1, 512, 512, pump_every=3)
        attention(2, 512, 512, pump_every=3)
        attention(3, 512, 512, pump_every=3)
        while filler_q:
            pump()
        # tile C (q 1024-2048): v chunks 2/3 are emitted at fixed blocks of
        # the first head (hard deps of pv blocks 8-15); o_proj of q 0-1024
        # is order-free filler spread across all four heads.
        filler_q.extend(
            (lambda nt=nt, mo=mo: o_proj(nt, mo))
            for nt in range(2) for mo in range(8))
        attention(0, 1024, 1024, at_blocks={
            5: [lambda: v_tb(2, 0), lambda: v_tb(2, 1)],
            7: [lambda: v_tb(2, 2), lambda: v_tb(2, 3)],
            9: [lambda: v_tb(3, 0), lambda: v_tb(3, 1)],
            11: [lambda: v_tb(3, 2), lambda: v_tb(3, 3)],
        })
        attention(1, 1024, 1024, pump_every=3, keep_warm=2)
        attention(2, 1024, 1024, pump_every=2, keep_warm=2)
        attention(3, 1024, 1024, pump_every=2, keep_warm=2)
        while filler_q:
            pump()
        # tail: q 1024-2048.  per mo one [128, 1024] ob filled in two
        # halves (nt=2 as soon as the early-closed psum bank is
        # normalized, nt=3 after the last), one merged DMA per mo.
        tail_obs = {}

        def tail_half(nt, mo):
            if mo not in tail_obs:
                tail_obs[mo] = obp.tile([128, 1024], F16, tag="obt",
                                        name="obt", bufs=8)
            ob = tail_obs[mo]
            ps = psum.tile([128, CH], F32, tag="qk", name="psob")
            for kb in range(2):
                nc.tensor.matmul(
                    ps[:, 0:512],
                    lhsT=wo_sb[kb][:, mo * 128:(mo + 1) * 128],
                    rhs=yT[kb][:, nt * 512:(nt + 1) * 512],
                    start=(kb == 0),
                    stop=(kb == 1),
                )
            half = nt - 2
            if mo % 2 == 1:
                nc.scalar.copy(ob[:, half * 512:(half + 1) * 512],
                               ps[:, 0:512])
            else:
                nc.vector.tensor_copy(
                    ob[:, half * 512:(half + 1) * 512], ps[:, 0:512])
            ring = nc.scalar if mo % 2 == 1 else nc.sync
            if mo >= 6:
                # last unit per queue: flush halves separately so the
                # final transfer is only 512 wide
                ring.dma_start(
                    out=outT_d[mo * 128:(mo + 1) * 128,
                               nt * 512:(nt + 1) * 512],
                    in_=ob[:, half * 512:(half + 1) * 512])
            elif nt == 3:
                ring.dma_start(
                    out=outT_d[mo * 128:(mo + 1) * 128, 1024:2048],
                    in_=ob[:])

        for mo in range(8):
            tail_half(2, mo)
        for mo in range(8):
            tail_half(3, mo)

    nc.compile()
    return nc


def shard_inputs(x, freqs_cos, freqs_sin, Wqkv, Wo):
    """Build the 8 per-core input maps (host-side sharding)."""
    x = np.asarray(x, dtype=np.float32)
    Wqkv = np.asarray(Wqkv, dtype=np.float32)
    Wo = np.asarray(Wo, dtype=np.float32)
    # cos/sin tables transposed and replicated x4 (one copy per local head)
    ccT = np.tile(np.asarray(freqs_cos, dtype=np.float32).T, (4, 1))
    ssT = np.tile(np.asarray(freqs_sin, dtype=np.float32).T, (4, 1))
    ccT = np.ascontiguousarray(ccT).astype(np.float16)
    ssT = np.ascontiguousarray(ssT).astype(np.float16)
    xTs = [np.ascontiguousarray(x[b].T).astype(np.float16) for b in range(B)]

    in_maps = []
    for c in range(NCORE):
        b, hg = c // 4, c % 4
        re = [np.arange(g * 64, g * 64 + 64, 2)
              for g in range(4 * hg, 4 * hg + 4)]
        im = [np.arange(g * 64 + 1, g * 64 + 64, 2)
              for g in range(4 * hg, 4 * hg + 4)]
        qcols = np.concatenate(re + im)
        kcols = C + qcols
        wqk = np.ascontiguousarray(
            Wqkv[:, np.concatenate([qcols, kcols])]).astype(np.float16)
        wv = np.ascontiguousarray(
            Wqkv[:, 2 * C + hg * 256: 2 * C + hg * 256 + 256]).astype(np.float16)
        wo = np.ascontiguousarray(
            Wo[hg * 256: hg * 256 + 256, :]).astype(np.float16)
        in_maps.append({
            "xT": xTs[b], "wqk": wqk, "wv": wv, "wo": wo,
            "ccT": ccT, "ssT": ssT,
        })
    return in_maps


_NC_CACHE = None


def _get_nc():
    global _NC_CACHE
    if _NC_CACHE is None:
        _NC_CACHE = build_nc()
    return _NC_CACHE


def run(inputs, trace=False):
    from concourse.bass_utils import run_bass_kernel_spmd

    nc = _get_nc()
    in_maps = shard_inputs(**inputs)
    res = run_bass_kernel_spmd(nc, in_maps, list(range(NCORE)), trace=trace)
    out = np.empty((B, T, C), dtype=np.float32)
    for b in range(B):
        acc = res.results[4 * b]["outT"].astype(np.float32)
        for c in range(4 * b + 1, 4 * b + 4):
            acc = acc + res.results[c]["outT"].astype(np.float32)
        out[b] = acc.T
    return out, res


def kernel(**inputs):
    out, _ = run(inputs)
    return out


# revision 58
# speedup vs baseline: 1.3118x; 1.0171x over previous
"""Causal self-attention (B=2, T=2048, C=1024, H=16, D=64) with RoPE on TRN2.

Sharding: 8 cores = 2 (batch) x 4 (head-groups of 4 heads).
Each core: qkv projection for its heads (fp16), RoPE, causal attention
(fp16 matmuls, fp32 psum), partial o_proj (row-parallel).  Host sums the
4 partial outputs per batch.

Layout is feature-major (xT etc.) so matmuls contract over partitions.
q/k features are de-interleaved on the host (re dims then im dims per
head) so RoPE is 4 full-width mults + sub/add per 128-row tile, then 8
small fp16 copies (4x DVE mode) relayout to head-contiguous qkT.

Attention computes S^T = (K Q^T) per 128-row k-block so probabilities
exit exp() already transposed for P^T @ V.  V tiles carry 64 all-ones
columns: the PV matmul then yields psum rows 0-63 = y, rows 64-127 = the
softmax denominator replicated 64x (zero extra PE cycles since matmul
cost is free-dim only) -- normalization is one DVE reciprocal + one DVE
multiply, no partition broadcast.  No max subtraction: logits are O(+-8)
and exp fits fp16 range.

q-tiles are staged 512/512/1024 so exp (ACT) starts as soon as chunk 0's
qkv is done; qkv chunks 2/3 and o_proj interleave into the attention
stream to keep PE busy while ACT chews exps.
"""

import sys
import os

sys.path.insert(0, "/opt/trn_rl_repo")

import numpy as np
from contextlib import ExitStack

import concourse.bass as bass
import concourse.bacc as bacc
import concourse.mybir as mybir
import concourse.tile as tile

F32 = mybir.dt.float32
F16 = mybir.dt.float16

# problem constants (hardcoded per contract)
B, T, C, NH, D = 2, 2048, 1024, 16, 64
HL = 4            # local heads per core
NCORE = 8
CH = 512          # qkv T-chunk width
NCHUNK = T // CH  # 4
SCALE = 1.0 / 8.0  # 1/sqrt(D)
NKB = T // 128    # 16 k-blocks


def _splits(a, b):
    """Split [a, b) at 512 boundaries (psum bank = 512 f32)."""
    out = []
    while a < b:
        nxt = min(b, (a // 512 + 1) * 512)
        out.append((a, nxt))
        a = nxt
    return out


def build_nc():
    nc = bacc.Bacc("TRN2", debug=False, num_devices=NCORE)

    xT_d = nc.dram_tensor("xT", [C, T], F16, kind="ExternalInput").ap()
    wqk_d = nc.dram_tensor("wqk", [C, 512], F16, kind="ExternalInput").ap()
    wv_d = nc.dram_tensor("wv", [C, 256], F16, kind="ExternalInput").ap()
    wo_d = nc.dram_tensor("wo", [256, C], F16, kind="ExternalInput").ap()
    ccT_d = nc.dram_tensor("ccT", [128, T], F16, kind="ExternalInput").ap()
    ssT_d = nc.dram_tensor("ssT", [128, T], F16, kind="ExternalInput").ap()
    outT_d = nc.dram_tensor("outT", [C, T], F16, kind="ExternalOutput").ap()

    with tile.TileContext(nc) as tc, ExitStack() as ctx:
        const = ctx.enter_context(tc.tile_pool(name="const", bufs=1))
        xcp = ctx.enter_context(tc.tile_pool(name="xcp", bufs=2))
        rtp = ctx.enter_context(tc.tile_pool(name="rtp", bufs=2))
        pp = ctx.enter_context(tc.tile_pool(name="pp", bufs=5))
        nrm = ctx.enter_context(tc.tile_pool(name="nrm", bufs=3))
        obp = ctx.enter_context(tc.tile_pool(name="obp", bufs=4))
        psum = ctx.enter_context(tc.tile_pool(name="psum", bufs=2, space="PSUM"))

        # ---- persistent SBUF tensors ----
        # wqk in two DMAs: q columns (m0/m1) land first so the first real
        # matmuls can start while the k half is still in flight.
        wqk_all = const.tile([128, 8 * 512], F16, tag="wqk", name="wqk")
        wqk_v = wqk_all[:].rearrange("p (kt n) -> p kt n", n=512)
        wqk_dv = wqk_d.rearrange("(kt p) n -> p kt n", p=128)
        nc.scalar.dma_start(out=wqk_v[:, :, 0:256], in_=wqk_dv[:, :, 0:256])
        nc.scalar.dma_start(out=wqk_v[:, :, 256:512], in_=wqk_dv[:, :, 256:512])
        wqk_sb = [wqk_all[:, kb * 512:(kb + 1) * 512] for kb in range(8)]

        # remaining input DMAs spread across issue queues so nothing
        # serializes behind the wqk/x loads
        cc = const.tile([128, T], F16, tag="cc")
        ss = const.tile([128, T], F16, tag="ss")
        nc.scalar.dma_start(out=cc[:], in_=ccT_d[:])
        nc.scalar.dma_start(out=ss[:], in_=ssT_d[:])

        wv_all = const.tile([128, 8 * 256], F16, tag="wv", name="wv")
        nc.scalar.dma_start(
            out=wv_all[:].rearrange("p (kt n) -> p kt n", n=256),
            in_=wv_d.rearrange("(kt p) n -> p kt n", p=128))
        wv_sb = [wv_all[:, kb * 256:(kb + 1) * 256] for kb in range(8)]

        # PE warm-up: dependency-free matmuls that cover the input-DMA wait
        # and carry the tensor engine through its p-state ramp before the
        # first real matmul issues.
        warm = const.tile([128, 512], F16, tag="warm")
        nc.gpsimd.memset(warm[:], 0.0)
        pw = psum.tile([128, 512], F32, tag="qk", name="pw")
        for _ in range(14):
            nc.tensor.matmul(
                pw[:, 0:512], lhsT=warm[:, 0:128], rhs=warm[:],
                start=True, stop=True)

        wo_all = const.tile([128, 2 * C], F16, tag="wo", name="wo")
        nc.sync.dma_start(
            out=wo_all[:].rearrange("p (kt n) -> p kt n", n=C),
            in_=wo_d.rearrange("(kt p) n -> p kt n", p=128))
        wo_sb = [wo_all[:, kb * C:(kb + 1) * C] for kb in range(2)]

        # qkT tiles: 0,1 = q (heads 01 / 23), 2,3 = k.  rows per tile:
        # [re_hA(32) im_hA(32) re_hB(32) im_hB(32)] after rope.
        qkT = [const.tile([128, T], F16, tag=f"qkT{m}", name=f"qkT{m}")
               for m in range(4)]
        # v: [128 kpos, 4 heads x 16 blocks x 128] fp16; cols 0-63 of each
        # block = v dims, cols 64-127 = ones (denominator rows of PV psum)
        vT = const.tile([128, HL * NKB * 128], F16, tag="vT", name="vT")
        vT_v = vT[:].rearrange("p (h b c) -> p h b c", h=HL, b=NKB)
        nc.gpsimd.memset(vT_v[:, :, :, 64:128], 1.0)
        # y^T tiles: [128, T] x2 (4 heads x 64 dims)
        yT = [const.tile([128, T], F16, tag=f"yT{kb}", name=f"yT{kb}")
              for kb in range(2)]

        chunk_xc = {}

        def x_load(n):
            t0 = n * CH
            xc_all = xcp.tile([128, 8 * CH], F16, tag="xc", name="xc")
            xc_view = xc_all[:].rearrange("p (kt t) -> p kt t", t=CH)
            xd_view = xT_d[:, t0:t0 + CH].rearrange("(kt p) t -> p kt t", p=128)
            nc.sync.dma_start(out=xc_view[:, 0:4], in_=xd_view[:, 0:4])
            nc.sync.dma_start(out=xc_view[:, 4:8], in_=xd_view[:, 4:8])
            chunk_xc[n] = [xc_all[:, kb * CH:(kb + 1) * CH] for kb in range(8)]

        def qk_g(n, g):
            """q (g=0) or k (g=1) projection + rope for chunk n."""
            t0 = n * CH
            xc = chunk_xc[n]
            mul = mybir.AluOpType.mult
            sub = mybir.AluOpType.subtract
            add = mybir.AluOpType.add
            pre = psum.tile([128, CH], F32, tag="qk", name="psre")
            pim = psum.tile([128, CH], F32, tag="qk", name="psim")
            for ps, m in ((pre, 2 * g), (pim, 2 * g + 1)):
                for kb in range(8):
                    nc.tensor.matmul(
                        ps[:, 0:CH],
                        lhsT=wqk_sb[kb][:, m * 128:(m + 1) * 128],
                        rhs=xc[kb],
                        start=(kb == 0),
                        stop=(kb == 7),
                    )
            ccn = cc[:, t0:t0 + CH]
            ssn = ss[:, t0:t0 + CH]
            t1 = rtp.tile([128, CH], F16, tag="t1")
            t2 = rtp.tile([128, CH], F16, tag="t2")
            t3 = rtp.tile([128, CH], F16, tag="t3")
            t4 = rtp.tile([128, CH], F16, tag="t4")
            # psum -> fp16 bounce on ACT (slack in the A/B phases; also
            # frees the psum slots quickly for the next projection); the
            # rope arithmetic then runs in DVE 2x fp16 mode.
            preb = rtp.tile([128, CH], F16, tag="preb")
            pimb = rtp.tile([128, CH], F16, tag="pimb")
            nc.scalar.copy(preb[:], pre[:, 0:CH])
            nc.scalar.copy(pimb[:], pim[:, 0:CH])
            preb, pimb = preb[:], pimb[:]
            nc.vector.tensor_tensor(t1[:], preb, ccn, mul)
            nc.vector.tensor_tensor(t2[:], pimb, ssn, mul)
            nc.vector.tensor_tensor(t3[:], preb, ssn, mul)
            nc.vector.tensor_tensor(t4[:], pimb, ccn, mul)
            rall, iall = t1, t3
            nc.vector.tensor_tensor(rall[:], t1[:], t2[:], sub)
            nc.vector.tensor_tensor(iall[:], t3[:], t4[:], add)
            # relayout: head h -> qkT[2*g + h//2] rows 64*(h%2)+[re|im]
            # fp16 sbuf-to-sbuf copies run in 4x DVE mode; Pool (idle in
            # these phases) takes a share to flatten the DVE bursts.
            for h in range(4):
                o = qkT[2 * g + h // 2]
                r0 = 64 * (h % 2)
                eng = nc.gpsimd if h == 3 else nc.vector
                eng.tensor_copy(
                    o[r0:r0 + 32, t0:t0 + CH], rall[32 * h:32 * h + 32, :])
                eng.tensor_copy(
                    o[r0 + 32:r0 + 64, t0:t0 + CH],
                    iall[32 * h:32 * h + 32, :])

        def v_tb(n, tb):
            xc = chunk_xc[n]
            psv = psum.tile([128, CH], F32, tag="qk", name="psv")
            for kb in range(8):
                nc.tensor.matmul(
                    psv[:, 0:256],
                    lhsT=xc[kb][:, tb * 128:(tb + 1) * 128],
                    rhs=wv_sb[kb],
                    start=(kb == 0),
                    stop=(kb == 7),
                )
            blk = 4 * n + tb
            dst = vT_v[:, :, blk, 0:64]
            src = psv[:, 0:256].rearrange("p (h d) -> p h d", d=64)
            # gpsimd cannot read PSUM; DVE does the psum->fp16 bounce
            nc.vector.tensor_copy(dst, src)

        filler_q = []

        def dummy(n=2):
            # keep-warm matmuls: PE p-state drops 2x after an idle gap and
            # needs 3us of continuous execution to recover; padding known
            # exp-bound stretches keeps the real matmuls at full clock.
            pd = psum.tile([128, 512], F32, tag="qk", name="pd")
            for _ in range(n):
                nc.tensor.matmul(
                    pd[:, 0:512], lhsT=warm[:, 0:128], rhs=warm[:],
                    start=True, stop=True)

        def pump(keep_warm=0):
            if filler_q:
                filler_q.pop(0)()
            elif keep_warm:
                dummy(keep_warm)

        def attention(h, q0, qn, pump_every=0, at_blocks=None,
                      keep_warm=0):
            """One head, q-cols [q0, q0+qn).  k-blocks 0..(q0+qn)/128.

            at_blocks: {kb: [unit, ...]} -- mandatory work units emitted
            just before scores_block(kb); used for dependencies of later
            pv_blocks (e.g. v tiles), unlike best-effort pump fillers.
            """
            qtile = qkT[h // 2]
            ktile = qkT[2 + h // 2]
            r0 = 64 * (h % 2)
            nkb = (q0 + qn) // 128
            fd = q0 // 128  # first diagonal block
            psy = psum.tile([128, qn], F32, tag="y", name="psy", bufs=1)
            Ps = {}

            def block_off(kb):
                return 128 * (kb - fd) if kb >= fd else 0

            def scores_block(kb):
                off = block_off(kb)
                pst = psum.tile([128, qn], F32, tag="st", name="pst")
                for (a, b) in _splits(off, qn):
                    nc.tensor.matmul(
                        pst[:, a:b],
                        lhsT=ktile[r0:r0 + 64, kb * 128:(kb + 1) * 128],
                        rhs=qtile[r0:r0 + 64, q0 + a:q0 + b],
                        start=True,
                        stop=True,
                    )
                P = pp.tile([128, 1024], F16, tag="P")
                Ps[kb] = P
                nc.scalar.activation(
                    P[:, off:qn], pst[:, off:qn],
                    mybir.ActivationFunctionType.Exp, scale=SCALE)
                if kb >= fd:
                    # zero strictly-upper triangle of the leading 128 cols
                    nc.gpsimd.affine_select(
                        out=P[:, off:off + 128],
                        in_=P[:, off:off + 128],
                        compare_op=mybir.AluOpType.is_ge,
                        fill=0.0,
                        base=0,
                        pattern=[[1, 128]],
                        channel_multiplier=-1,
                    )

            def pv_block(kb):
                off = block_off(kb)
                P = Ps.pop(kb)
                for (a, b) in _splits(off, qn):
                    # last writer of the psum bank holding col a is diag
                    # block fd + 4*(a//512) + 3
                    kb_stop = min(fd + 4 * (a // 512) + 3, nkb - 1)
                    nc.tensor.matmul(
                        psy[:, a:b],
                        lhsT=vT_v[:, h, kb, :],
                        rhs=P[:, a:b],
                        start=(kb == 0),
                        stop=(kb == kb_stop),
                    )

            def normalize(a, b):
                # psum rows 64-127 all hold the denominator row l (ones
                # cols of vT): reciprocal + one multiply per psum bank,
                # emitted as soon as that bank's accumulation closes.
                rlb = nrm.tile([64, 512], F32, tag="rlb")
                nc.vector.reciprocal(rlb[:, 0:b - a], psy[64:128, a:b])
                nc.vector.tensor_tensor(
                    yT[h // 2][r0:r0 + 64, q0 + a:q0 + b],
                    psy[0:64, a:b], rlb[:, 0:b - a], mybir.AluOpType.mult)

            # scores run one block ahead of PV so PE is not stalled on exp
            for kb in range(nkb):
                if at_blocks and kb in at_blocks:
                    for u in at_blocks[kb]:
                        u()
                if pump_every and kb % pump_every == 0:
                    pump(keep_warm)
                scores_block(kb)
                if kb > 0:
                    pv_block(kb - 1)
                    if kb - 1 == min(fd + 3, nkb - 1) and qn > 512:
                        normalize(0, 512)  # bank 0 closed early
            pv_block(nkb - 1)
            normalize(512 if qn > 512 else 0, qn)

        def o_proj(nt, mo, tail=False):
            """Output block: feat rows [128*mo ..+128), q [512*nt ..+512)."""
            ob = obp.tile([128, 512], F16, tag="ob", name="ob")
            ps = psum.tile([128, CH], F32, tag="qk", name="psob")
            for kb in range(2):
                nc.tensor.matmul(
                    ps[:, 0:512],
                    lhsT=wo_sb[kb][:, mo * 128:(mo + 1) * 128],
                    rhs=yT[kb][:, nt * 512:(nt + 1) * 512],
                    start=(kb == 0),
                    stop=(kb == 1),
                )
            # in the tail ACT is idle once the exps are done: it takes half
            # the psum bounces there
            if tail and mo % 2 == 1:
                nc.scalar.copy(ob[:], ps[:, 0:512])
            else:
                nc.vector.tensor_copy(ob[:], ps[:, 0:512])
            # keep DMA issue off the ACT queue while exps run; in the tail
            # ACT is free and a second queue doubles drain bandwidth
            ring = nc.scalar if (tail and mo % 2 == 1) else nc.sync
            ring.dma_start(
                out=outT_d[mo * 128:(mo + 1) * 128, nt * 512:(nt + 1) * 512],
                in_=ob[:])

        # ---- schedule ----
        # Emission order == per-engine queue order.  Attention is the only
        # ACT (exp) consumer and PE outpaces ACT ~2:1 there, so qkv/o_proj
        # work is pumped into the attention stream as PE filler at k-block
        # granularity.  Dependency safety comes from emission order: a
        # filler is always emitted before the instruction that needs it.
        x_load(0)
        x_load(1)
        qk_g(0, 0)
        qk_g(0, 1)
        for tb in range(4):
            v_tb(0, tb)
        v_tb(1, 0)
        v_tb(1, 1)
        qk_g(1, 0)
        # tile A (q 0-512): needs only chunk 0.  fillers: rest of chunk 1.
        filler_q.extend([
            lambda: qk_g(1, 1),
            lambda: v_tb(1, 2),
            lambda: v_tb(1, 3),
        ])
        attention(0, 0, 512, pump_every=2)
        attention(1, 0, 512, pump_every=2)
        attention(2, 0, 512, pump_every=2)
        attention(3, 0, 512, pump_every=2)
        while filler_q:
            pump()
        # tile B (q 512-1024): fillers: chunks 2,3 qk.
        x_load(2)
        qk_g(2, 0)
        filler_q.extend([
            lambda: qk_g(2, 1),
            lambda: x_load(3),
            lambda: qk_g(3, 0),
            lambda: qk_g(3, 1),
        ])
        attention(0, 512, 512, pump_every=3)
        attention(1, 512, 512, pump_every=3)
        attention(2, 512, 512, pump_every=3)
        attention(3, 512, 512, pump_every=3)
        while filler_q:
            pump()
        # tail o_proj helpers (q 1024-2048): per mo one [128, 1024] ob
        # filled in two halves; nt=2 halves are emitted inside the last
        # head's attention as soon as its early psum bank is normalized.
        tail_obs = {}

        def tail_half(nt, mo):
            if mo not in tail_obs:
                tail_obs[mo] = obp.tile([128, 1024], F16, tag="obt",
                                        name="obt", bufs=8)
            ob = tail_obs[mo]
            ps = psum.tile([128, CH], F32, tag="qk", name="psob")
            for kb in range(2):
                nc.tensor.matmul(
                    ps[:, 0:512],
                    lhsT=wo_sb[kb][:, mo * 128:(mo + 1) * 128],
                    rhs=yT[kb][:, nt * 512:(nt + 1) * 512],
                    start=(kb == 0),
                    stop=(kb == 1),
                )
            half = nt - 2
            if mo % 2 == 1:
                nc.scalar.copy(ob[:, half * 512:(half + 1) * 512],
                               ps[:, 0:512])
            else:
                nc.vector.tensor_copy(
                    ob[:, half * 512:(half + 1) * 512], ps[:, 0:512])
            ring = nc.scalar if mo % 2 == 1 else nc.sync
            if mo >= 6:
                # last unit per queue: flush halves separately so the
                # final transfer is only 512 wide
                ring.dma_start(
                    out=outT_d[mo * 128:(mo + 1) * 128,
                               nt * 512:(nt + 1) * 512],
                    in_=ob[:, half * 512:(half + 1) * 512])
            elif nt == 3:
                ring.dma_start(
                    out=outT_d[mo * 128:(mo + 1) * 128, 1024:2048],
                    in_=ob[:])

        # tile C (q 1024-2048): v chunks 2/3 are emitted at fixed blocks of
        # the first head (hard deps of pv blocks 8-15); o_proj of q 0-1024
        # is order-free filler spread across all four heads.
        filler_q.extend(
            (lambda nt=nt, mo=mo: o_proj(nt, mo))
            for nt in range(2) for mo in range(8))
        attention(0, 1024, 1024, at_blocks={
            5: [lambda: v_tb(2, 0), lambda: v_tb(2, 1)],
            7: [lambda: v_tb(2, 2), lambda: v_tb(2, 3)],
            9: [lambda: v_tb(3, 0), lambda: v_tb(3, 1)],
            11: [lambda: v_tb(3, 2), lambda: v_tb(3, 3)],
        })
        attention(1, 1024, 1024, pump_every=3, keep_warm=2)
        attention(2, 1024, 1024, pump_every=2, keep_warm=2)
        attention(3, 1024, 1024, pump_every=2, keep_warm=2, at_blocks={
            13: [lambda mo=mo: tail_half(2, mo) for mo in range(4)],
            15: [lambda mo=mo: tail_half(2, mo) for mo in range(4, 8)],
        })
        while filler_q:
            pump()
        for mo in range(8):
            tail_half(3, mo)

    nc.compile()
    return nc


def shard_inputs(x, freqs_cos, freqs_sin, Wqkv, Wo):
    """Build the 8 per-core input maps (host-side sharding)."""
    x = np.asarray(x, dtype=np.float32)
    Wqkv = np.asarray(Wqkv, dtype=np.float32)
    Wo = np.asarray(Wo, dtype=np.float32)
    # cos/sin tables transposed and replicated x4 (one copy per local head)
    ccT = np.tile(np.asarray(freqs_cos, dtype=np.float32).T, (4, 1))
    ssT = np.tile(np.asarray(freqs_sin, dtype=np.float32).T, (4, 1))
    ccT = np.ascontiguousarray(ccT).astype(np.float16)
    ssT = np.ascontiguousarray(ssT).astype(np.float16)
    xTs = [np.ascontiguousarray(x[b].T).astype(np.float16) for b in range(B)]

    in_maps = []
    for c in range(NCORE):
        b, hg = c // 4, c % 4
        re = [np.arange(g * 64, g * 64 + 64, 2)
              for g in range(4 * hg, 4 * hg + 4)]
        im = [np.arange(g * 64 + 1, g * 64 + 64, 2)
              for g in range(4 * hg, 4 * hg + 4)]
        qcols = np.concatenate(re + im)
        kcols = C + qcols
        wqk = np.ascontiguousarray(
            Wqkv[:, np.concatenate([qcols, kcols])]).astype(np.float16)
        wv = np.ascontiguousarray(
            Wqkv[:, 2 * C + hg * 256: 2 * C + hg * 256 + 256]).astype(np.float16)
        wo = np.ascontiguousarray(
            Wo[hg * 256: hg * 256 + 256, :]).astype(np.float16)
        in_maps.append({
            "xT": xTs[b], "wqk": wqk, "wv": wv, "wo": wo,
            "ccT": ccT, "ssT": ssT,
        })
    return in_maps


_NC_CACHE = None


def _get_nc():
    global _NC_CACHE
    if _NC_CACHE is None:
        _NC_CACHE = build_nc()
    return _NC_CACHE


def run(inputs, trace=False):
    from concourse.bass_utils import run_bass_kernel_spmd

    nc = _get_nc()
    in_maps = shard_inputs(**inputs)
    res = run_bass_kernel_spmd(nc, in_maps, list(range(NCORE)), trace=trace)
    out = np.empty((B, T, C), dtype=np.float32)
    for b in range(B):
        acc = res.results[4 * b]["outT"].astype(np.float32)
        for c in range(4 * b + 1, 4 * b + 4):
            acc = acc + res.results[c]["outT"].astype(np.float32)
        out[b] = acc.T
    return out, res


def kernel(**inputs):
    out, _ = run(inputs)
    return out
